# revision 1
# baseline (speedup 1.0000x reference)
"""CARAFE content-aware upsampling on 8 Trainium2 NeuronCores (Bass/Tile).

Problem: x[2,256,64,64], 1x1 compress conv (256->32), 5x5 encoder conv
(32->100), pixel-shuffle(r=2) + softmax over 25 taps, then dynamic-filter
reassembly: out[b,c,2h+r1,2w+r2] = sum_k x[b,c,h+di,w+dj] * softmax_w.

Sharding: pure data-parallel over (batch, 16-row H slices) -> 8 cores.
Each core receives its zero-padded input slice (halo rows pre-padded in
numpy) and computes a [256, 32, 128] output slice.

Per-core mapping:
  - PE transposes the x slice into [w_padded, (row, c)] layout; the MAC
    stationaries (overlapping 6x20 windows) are gathered by DMA early so
    they overlap the conv phase.
  - compress conv (1x1) and encoder conv (5x5, as 25 PSUM-accumulated
    matmuls over shifted y1 views) run on PE, split by output row parity
    so the result columns come out in scatter-friendly (w, tile, b4) order.
  - softmax stays channel-major: tap-sums and the reciprocal broadcast are
    tiny select-matrix matmuls on PE; normalize is one DVE multiply.
  - The 25-tap dynamic-filter sum runs on PE as dense [120x128]x[120x128]
    matmuls against block-sparse band matrices; the normalized weights are
    scattered into the bands by 160 per-(parity, di, w) DMAs (walrus
    requires dim0 of an SBUF DMA AP to stride whole partitions, so the
    band diagonal is decomposed per output column w).
  - DMA dispatch is spread across the SP/ACT HWDGE queues and the Pool
    SWDGE queue to balance engine occupancy.
"""

import sys

sys.path.insert(0, "/opt/trn_rl_repo")

import numpy as np

import concourse.bacc as bacc
import concourse.bass as bass
import concourse.tile as tile
from concourse import mybir
from concourse.ap import AP

F32 = mybir.dt.float32

# geometry
B, C, H, W = 2, 256, 64, 64
RATIO, K_UP, C_MID, ENC_K = 2, 5, 32, 5
NK = RATIO * RATIO * K_UP * K_UP  # 100
HSLICE = 16                       # output source rows per core
ROWS = HSLICE + 4                 # with 2-row halo each side
WP = W + 4                        # padded width
PADPOS = ROWS * WP                # 1360
NPOS = HSLICE * W                 # 1024
NCORES = 8

# MAC blocking: 2 source rows x 16 source cols per block
BLK_W = 16
BLK_N = 2 * BLK_W * 4            # 128 outputs per block
KDIM = 6 * 20                    # 120 window pixels per block
NBLK = (HSLICE // 2) * (W // BLK_W)  # 8 row-pairs * 4 = 32
YF = NBLK * BLK_N                # 4096 free dim of Y-big


def build_program(with_ebias: bool):
    nc = bacc.Bacc()
    xs_d = nc.declare_dram_parameter("xs", [2, 128, PADPOS], F32, isOutput=False)
    wct_d = nc.declare_dram_parameter("wct", [2, 128, C_MID], F32, isOutput=False)
    wet_d = nc.declare_dram_parameter("wet32", [C_MID, 25 * NK], F32, isOutput=False)
    ident_d = nc.declare_dram_parameter("ident", [128, 128], F32, isOutput=False)
    sel_d = nc.declare_dram_parameter("sel", [NK, 4], F32, isOutput=False)
    selt_d = nc.declare_dram_parameter("selt", [4, NK], F32, isOutput=False)
    if with_ebias:
        ebias_d = nc.declare_dram_parameter("ebias", [2, NK, 512], F32, isOutput=False)
    out_d = nc.declare_dram_parameter("out", [2, 128, 32 * 128], F32, isOutput=True)

    with tile.TileContext(nc) as tc:
        # The byte-range race detector cannot model the diagonal scatter
        # APs (partition+free coupled strides) and reports false positives;
        # dependency generation itself is tensor-granular and conservative,
        # and every raw-AP tensor here is persistent (no slot reuse).
        tc.race_detector_enabled = False
        with (
            tc.tile_pool(name="persist", bufs=1) as pp,
            tc.tile_pool(name="psTP", bufs=1, space="PSUM") as psTP,
            tc.tile_pool(name="psCMP", bufs=1, space="PSUM") as psCMP,
            tc.tile_pool(name="psENC", bufs=1, space="PSUM") as psENC,
            tc.tile_pool(name="psSM", bufs=1, space="PSUM") as psSM,
            tc.tile_pool(name="psMAC", bufs=3, space="PSUM") as psMAC,
        ):
            ident = pp.tile([128, 128], F32, tag="ident")
            nc.sync.dma_start(ident[:], ident_d[:])
            sel = pp.tile([NK, 4], F32, tag="sel")
            nc.sync.dma_start(sel[:], sel_d[:])
            selt = pp.tile([4, NK], F32, tag="selt")
            nc.sync.dma_start(selt[:], selt_d[:])

            xin = []
            for ct in range(2):
                t = pp.tile([128, PADPOS], F32, tag=f"xin{ct}")
                nc.sync.dma_start(t[:], xs_d[ct])
                xin.append(t)

            wct = []
            for ct in range(2):
                t = pp.tile([128, C_MID], F32, tag=f"wct{ct}")
                nc.sync.dma_start(t[:], wct_d[ct])
                wct.append(t)

            wetb = pp.tile([C_MID, 25 * NK], F32, tag="wetb")
            nc.sync.dma_start(wetb[:], wet_d[:])

            if with_ebias:
                ebias = []
                for ro in range(2):
                    t = pp.tile([NK, 512], F32, name=f"ebias{ro}", tag=f"ebias{ro}")
                    nc.sync.dma_start(t[:], ebias_d[ro])
                    ebias.append(t)

            # ---- phase 1: transpose x into xT [WP, (row, c)] ----
            xT = pp.tile([WP, ROWS * C], F32, tag="xT")
            for r in range(ROWS):
                for ct in range(2):
                    ps = psTP.tile([WP, 128], F32, tag="tp")
                    nc.tensor.transpose(
                        ps[:], xin[ct][:, r * WP:(r + 1) * WP], ident[:]
                    )
                    eng = nc.vector if (r * 2 + ct) % 2 == 0 else nc.scalar
                    if eng is nc.vector:
                        eng.tensor_copy(
                            xT[:, r * C + ct * 128: r * C + ct * 128 + 128], ps[:]
                        )
                    else:
                        eng.copy(
                            xT[:, r * C + ct * 128: r * C + ct * 128 + 128], ps[:]
                        )

            # ---- phase 1b: gather MAC stationaries (overlaps conv phase) ----
            xcs = []
            nq = 0
            for g in range(8):
                xc = pp.tile([KDIM, 4 * C], F32, name=f"xc{g}", tag=f"xc{g}")
                for r in range(6):
                    for b4 in range(4):
                        eng = (nc.sync, nc.scalar, nc.sync, nc.scalar,
                               nc.gpsimd, nc.sync, nc.scalar, nc.gpsimd)[g]
                        eng.dma_start(
                            AP(xc.tensor, r * 20 * (4 * C) + b4 * C,
                               [[4 * C, 20], [1, C]]),
                            AP(xT.tensor,
                               (2 * g + r) * C + b4 * 16 * (ROWS * C),
                               [[ROWS * C, 20], [1, C]]),
                        )
                xcs.append(xc)

            # ---- phase 2: compress conv y1[32, PADPOS] ----
            y1 = pp.tile([C_MID, PADPOS], F32, tag="y1")
            off = 0
            while off < PADPOS:
                n = min(512, PADPOS - off)
                ps = psCMP.tile([C_MID, 512], F32, tag="cmp")
                nc.tensor.matmul(
                    ps[:, :n], wct[0][:], xin[0][:, off:off + n],
                    start=True, stop=False,
                )
                nc.tensor.matmul(
                    ps[:, :n], wct[1][:], xin[1][:, off:off + n],
                    start=False, stop=True,
                )
                nc.vector.tensor_copy(y1[:, off:off + n], ps[:, :n])
                off += n

            # ---- phase 4: encoder conv, split by row-parity ro ----
            # rhs columns stream in pos' = (w, tile, b4) order so that
            # (tile, b4) is contiguous in the result -> scatter-friendly.
            # ---- phase 5: softmax in channel-major layout ----
            #   sums over the 25 taps per sub via a [100,4] select matmul,
            #   reciprocal, broadcast back via [4,100] matmul, multiply.
            yM = []
            for ro in range(2):
                ps = psENC.tile([NK, 512], F32, tag="enc")
                for tap in range(25):
                    di, dj = tap // 5 - 2, tap % 5 - 2
                    rhs = AP(
                        y1.tensor,
                        (ro + di + 2) * WP + dj + 2,
                        [[PADPOS, C_MID], [1, 16], [2 * WP, 8], [16, 4]],
                    )
                    nc.tensor.matmul(
                        ps[:], wetb[:, tap * NK:(tap + 1) * NK], rhs,
                        start=(tap == 0), stop=(tap == 24),
                    )
                y2e = pp.tile([NK, 512], F32, name=f"y2e{ro}", tag=f"y2e{ro}")
                if with_ebias:
                    nc.vector.scalar_tensor_tensor(
                        y2e[:], ps[:], 1.0, ebias[ro][:],
                        op0=mybir.AluOpType.mult, op1=mybir.AluOpType.add,
                    )
                else:
                    nc.vector.tensor_copy(y2e[:], ps[:])
                nc.scalar.activation(
                    y2e[:], y2e[:], mybir.ActivationFunctionType.Exp
                )
                pss = psSM.tile([4, 512], F32, tag="sums")
                nc.tensor.matmul(pss[:], sel[:], y2e[:], start=True, stop=True)
                rsum4 = pp.tile([4, 512], F32, name=f"rsum4{ro}", tag=f"rsum4{ro}")
                nc.vector.reciprocal(rsum4[:], pss[:])
                psb = psSM.tile([NK, 512], F32, tag="bcast")
                nc.tensor.matmul(psb[:], selt[:], rsum4[:], start=True, stop=True)
                t = pp.tile([NK, 512], F32, name=f"yM{ro}", tag=f"yM{ro}")
                nc.vector.tensor_tensor(
                    t[:], y2e[:], psb[:], op=mybir.AluOpType.mult
                )
                yM.append(t)

            # ---- phase 7: scatter into band matrices ----
            # ybig column layout: n = ((ro*16 + w)*4 + sub)*32 + tb, so each
            # per-(ro,dii,w) DMA is [[512,20],[1,32]] -> [[YF,5],[32,4],[1,32]]
            osbs = [pp.tile([128, 512], F32, name=f"osb{i}", tag=f"osb{i}")
                    for i in range(4)]
            ybig = pp.tile([KDIM, YF], F32, tag="ybig")
            for p0 in range(0, KDIM, 32):
                nc.gpsimd.memset(ybig[p0:min(p0 + 32, KDIM), :], 0.0)
            nq2 = 0
            for ro in range(2):
                for dii in range(5):
                    eng = (nc.gpsimd, nc.scalar, nc.sync, nc.gpsimd, nc.scalar,
                           nc.sync, nc.gpsimd, nc.scalar, nc.gpsimd, nc.sync)[ro * 5 + dii]
                    for w in range(16):
                        src = AP(yM[ro].tensor, (dii * 20) * 512 + w * 32,
                                 [[512, 20], [1, 32]])
                        dst = AP(
                            ybig.tensor,
                            ((ro + dii) * 20 + w) * YF + (ro * 16 + w) * 128,
                            [[YF, 5], [32, 4], [1, 32]],
                        )
                        eng.dma_start(dst, src)

            # ---- phases 8-10: per row-pair: MAC matmuls, store ----
            for g in range(8):          # row-pair groups
                xc = xcs[g]
                for ct in range(2):
                    ps = psMAC.tile([128, 512], F32, tag="mac")
                    for b4 in range(4):
                        blk = g * 4 + b4
                        nc.tensor.matmul(
                            ps[:, b4 * 128:(b4 + 1) * 128],
                            xc[:, b4 * C + ct * 128:b4 * C + ct * 128 + 128],
                            AP(ybig.tensor, blk, [[YF, KDIM], [32, 128]]),
                            start=True, stop=True,
                        )
                    osb = osbs[(g * 2 + ct) % 4]
                    # keep psum's natural col order (b4, ro, w, sub); the
                    # numpy unshard permutes to output row order on CPU.
                    if ct == 0:
                        nc.vector.tensor_copy(osb[:], ps[:])
                    else:
                        nc.scalar.copy(osb[:], ps[:])
                    oeng = nc.scalar if (g + ct) % 2 == 0 else nc.sync
                    oeng.dma_start(
                        out_d[ct, :, g * 512:(g + 1) * 512], osb[:]
                    )
    nc.compile()
    return nc


_CACHE: dict[bool, object] = {}


def _get_program(with_ebias: bool):
    if with_ebias not in _CACHE:
        _CACHE[with_ebias] = build_program(with_ebias)
    return _CACHE[with_ebias]


def _prep_inputs(x, w_comp, b_comp, w_enc, b_enc):
    """Build the per-core numpy input dicts."""
    x = np.asarray(x, dtype=np.float32)
    w_comp = np.asarray(w_comp, dtype=np.float32)
    b_comp = np.asarray(b_comp, dtype=np.float32)
    w_enc = np.asarray(w_enc, dtype=np.float32)
    b_enc = np.asarray(b_enc, dtype=np.float32)

    # weights, replicated
    wct = np.ascontiguousarray(
        w_comp.T.reshape(2, 128, C_MID)
    )
    # wet32[m, (tap, o)]: per-tap [32, 100] stationaries
    we = w_enc.reshape(NK, C_MID, 25)           # [o, m, tap]
    wet32 = np.ascontiguousarray(
        np.transpose(we, (1, 2, 0)).reshape(C_MID, 25 * NK)
    )
    ident = np.eye(128, dtype=np.float32)
    sel = np.zeros((NK, 4), dtype=np.float32)
    sel[np.arange(NK), np.arange(NK) % 4] = 1.0
    selt = np.ascontiguousarray(sel.T)

    # encoder bias field (b_enc + conv of b_comp over valid mask), per slice
    with_ebias = bool(b_comp.any() or b_enc.any())

    in_maps = []
    for core in range(NCORES):
        b = core // 4
        h0 = (core % 4) * HSLICE
        xs = np.zeros((C, ROWS, WP), dtype=np.float32)
        r_lo = max(0, h0 - 2)
        r_hi = min(H, h0 + HSLICE + 2)
        xs[:, (r_lo - (h0 - 2)):(r_hi - (h0 - 2)), 2:2 + W] = x[b, :, r_lo:r_hi, :]
        m = {
            "xs": np.ascontiguousarray(
                xs.reshape(2, 128, ROWS, WP).reshape(2, 128, PADPOS)
            ),
            "wct": wct,
            "wet32": wet32,
            "ident": ident,
            "sel": sel,
            "selt": selt,
        }
        if with_ebias:
            # field[o, h, w] = b_enc[o] + sum_m sum_taps_valid w_enc[o,m,tap] b_comp[m]
            wb = np.einsum("omt,m->ot", we, b_comp).reshape(NK, 5, 5)
            field = np.zeros((NK, HSLICE, W), dtype=np.float32)
            for di in range(-2, 3):
                for dj in range(-2, 3):
                    hh = np.arange(h0, h0 + HSLICE)[:, None] + di
                    ww = np.arange(W)[None, :] + dj
                    valid = ((hh >= 0) & (hh < H) & (ww >= 0) & (ww < W))
                    field += (
                        wb[:, di + 2, dj + 2][:, None, None]
                        * valid[None].astype(np.float32)
                    )
            field += b_enc[:, None, None]
            # per-ro, columns in pos' = (w, tile, b4) order
            f = field.reshape(NK, 8, 2, 4, 16)        # (o, tile, ro, b4, w)
            f = np.transpose(f, (2, 0, 4, 1, 3))      # (ro, o, w, tile, b4)
            m["ebias"] = np.ascontiguousarray(f.reshape(2, NK, 512))
        in_maps.append(m)
    return in_maps, with_ebias


TRACE = False
LAST_RESULT = None


def kernel(x, w_comp, b_comp, w_enc, b_enc):
    global LAST_RESULT
    from concourse.bass_utils import run_bass_kernel_spmd

    in_maps, with_ebias = _prep_inputs(x, w_comp, b_comp, w_enc, b_enc)
    nc = _get_program(with_ebias)
    res = run_bass_kernel_spmd(
        nc, in_maps, core_ids=list(range(NCORES)), trace=TRACE
    )
    LAST_RESULT = res
    out = np.empty((B, C, 2 * H, 2 * W), dtype=np.float32)
    for core in range(NCORES):
        b = core // 4
        h0 = (core % 4) * HSLICE
        o = res.results[core]["out"].reshape(2, 128, 8, 4, 2, 16, 2, 2)
        # axes: (ct, c, g, b4, ro, w, r1, r2) -> (ct, c, g, ro, r1, b4, w, r2)
        o = np.transpose(o, (0, 1, 2, 4, 6, 3, 5, 7)).reshape(2, 128, 32, 128)
        out[b, :128, 2 * h0:2 * h0 + 32, :] = o[0]
        out[b, 128:, 2 * h0:2 * h0 + 32, :] = o[1]
    return out



# revision 8
# speedup vs baseline: 2.7423x; 2.7423x over previous
"""CARAFE content-aware upsampling on 8 Trainium2 NeuronCores (Bass/Tile).

Problem: x[2,256,64,64], 1x1 compress conv (256->32), 5x5 encoder conv
(32->100), pixel-shuffle(r=2) + softmax over 25 taps, then dynamic-filter
reassembly: out[b,c,2h+r1,2w+r2] = sum_k x[b,c,h+di,w+dj] * softmax_w.

Sharding: pure data-parallel over (batch, 16-row H slices) -> 8 cores.

Per-core pipeline (DMA-dispatch-minimal redesign):
  - compress (1x1) and encoder (5x5 as 25 PSUM-accumulated matmuls over
    shifted y1 views) run on PE in float32r (1 cyc/row at free>=256).
  - softmax stays channel-major; a per-sub select matmul then regroups
    the 100 = (25 taps x 4 subpixels) partition layout to taps-only
    partitions with sub on the free axis, folding in the 1/sum
    normalization (PE broadcast matmul + one DVE multiply per sub).
  - the banded 25-tap MAC operand is built via a DRAM round trip: the
    diagonal (partition+free coupled) strides live entirely on the DRAM
    side, so the whole scatter is 10 large DMAs into a pre-zeroed DRAM
    scratch + 1 dense load back (vs 160 per-column SBUF scatters).
  - x windows arrive pre-transposed from the host (xt2, bf16) and are
    gathered into the [120, g*b4*c] MAC stationary by 6 SWDGE DMAs.
  - the MAC runs in bf16 (1 cyc/row): 64 matmuls [120]x[128], psum
    [128c, 128n], results stored bf16 and upcast on the host.
"""

import sys

sys.path.insert(0, "/opt/trn_rl_repo")

import numpy as np

import concourse.bacc as bacc
import concourse.bass as bass
import concourse.tile as tile
from concourse import mybir
from concourse.ap import AP

F32 = mybir.dt.float32
F32R = mybir.dt.float32r
BF16 = mybir.dt.bfloat16

# geometry
B, C, H, W = 2, 256, 64, 64
RATIO, K_UP, C_MID, ENC_K = 2, 5, 32, 5
NK = RATIO * RATIO * K_UP * K_UP  # 100
HSLICE = 16                       # output source rows per core
ROWS = HSLICE + 4                 # with 2-row halo each side
WP = W + 4                        # padded width
PADPOS = ROWS * WP                # 1360
NCORES = 8

KDIM = 120                        # 6 rows x 20 cols window pixels
YF = 4096                         # ybig free dim: col = 32*n + blk
XCF = 8192                        # xcall free dim: (g, b4, c)
YMSF = 2048                       # yMs free dim: (w, sub, tb)
WPKW = 268                        # wpk cols: wcat 64 | sel 4 | Esel 100 | esub 100


def build_program(with_ebias: bool):
    nc = bacc.Bacc()
    xs_d = nc.declare_dram_parameter("xs", [2, 128, PADPOS], BF16, isOutput=False)
    xt2_d = nc.declare_dram_parameter("xt2", [ROWS, 20, 4, C], BF16, isOutput=False)
    wpk_d = nc.declare_dram_parameter("wpk", [128, WPKW], BF16, isOutput=False)
    wet_d = nc.declare_dram_parameter("wet32", [C_MID, 25 * NK], BF16, isOutput=False)
    if with_ebias:
        ebias_d = nc.declare_dram_parameter("ebias", [2, NK, 512], F32, isOutput=False)
    # band scratch; ExternalOutput buffers are pre-zeroed by the runtime,
    # which supplies the structural zeros between the band diagonals.
    ydram_d = nc.declare_dram_parameter("ydram", [KDIM, YF], BF16, isOutput=True)
    out_d = nc.declare_dram_parameter("out", [2, 128, 4096], BF16, isOutput=True)

    with tile.TileContext(nc) as tc:
        # The diagonal DRAM scatter APs (partition+free coupled strides on
        # the DRAM side) confuse the byte-range race detector; dependency
        # generation itself is tensor-granular and conservative.
        tc.race_detector_enabled = False
        with (
            tc.tile_pool(name="persist", bufs=1) as pp,
            tc.tile_pool(name="psCMP", bufs=1, space="PSUM") as psCMP,
            tc.tile_pool(name="psENC", bufs=2, space="PSUM") as psENC,
            tc.tile_pool(name="psS", bufs=1, space="PSUM") as psS,
            tc.tile_pool(name="psRG", bufs=1, space="PSUM") as psRG,
            tc.tile_pool(name="psB", bufs=1, space="PSUM") as psB,
            tc.tile_pool(name="psMAC", bufs=2, space="PSUM") as psMAC,
        ):
            # ---- input loads ----
            xst = []
            for ct in range(2):
                t = pp.tile([128, PADPOS], BF16, tag=f"xs{ct}")
                nc.sync.dma_start(t[:], xs_d[ct])
                xst.append(t)

            wpk = pp.tile([128, WPKW], BF16, tag="wpk")
            nc.scalar.dma_start(wpk[:], wpk_d[:])
            wetb = pp.tile([C_MID, 25 * NK], BF16, tag="wetb")
            nc.scalar.dma_start(wetb[:], wet_d[:])

            if with_ebias:
                ebias = []
                for ro in range(2):
                    t = pp.tile([NK, 512], F32, tag=f"ebias{ro}")
                    nc.scalar.dma_start(t[:], ebias_d[ro])
                    ebias.append(t)

            # xcall[rq*20+cq, (g, b4, c)] = x[c, 2g+rq, b4*16+cq] (padded
            # coords), gathered from the host-transposed xt2 on the SWDGE
            # queue (Pool engine) to keep HWDGE free.
            xcall = pp.tile([KDIM, XCF], BF16, tag="xcall")
            for rq in range(6):
                dst = AP(xcall.tensor, rq * 20 * XCF,
                         [[XCF, 20], [1024, 8], [1, 1024]])
                src = AP(xt2_d, rq * 20480,
                         [[1024, 20], [40960, 8], [1, 1024]])
                nc.gpsimd.dma_start(dst, src)

            # ---- compress conv y1[32, PADPOS] (f32r) ----
            y1 = pp.tile([C_MID, PADPOS], BF16, tag="y1")
            off = 0
            ci = 0
            while off < PADPOS:
                n = min(512, PADPOS - off)
                ps = psCMP.tile([C_MID, 512], F32, tag="cmp")
                for ct in range(2):
                    nc.tensor.matmul(
                        ps[:, :n],
                        wpk[:, ct * 32:(ct + 1) * 32],
                        xst[ct][:, off:off + n],
                        start=(ct == 0), stop=(ct == 1),
                    )
                eng = nc.vector if ci % 2 == 0 else nc.scalar
                if eng is nc.vector:
                    eng.tensor_copy(y1[:, off:off + n], ps[:, :n])
                else:
                    eng.copy(y1[:, off:off + n], ps[:, :n])
                off += n
                ci += 1

            # ---- encoder conv + exp, per output-row parity ro ----
            # rhs columns stream in pos' = (w, tile, b4) order:
            # f = 32*w + 4*tile + b4  <->  (h = 2*tile + ro, wcol = 16*b4 + w)
            def encode(ro):
                ps = psENC.tile([NK, 512], F32, tag="enc")
                for tap in range(25):
                    di, dj = tap // 5 - 2, tap % 5 - 2
                    rhs = AP(
                        y1.tensor,
                        (ro + di + 2) * WP + dj + 2,
                        [[PADPOS, C_MID], [1, 16], [2 * WP, 8], [16, 4]],
                    )
                    nc.tensor.matmul(
                        ps[:], wetb[:, tap * NK:(tap + 1) * NK],
                        rhs,
                        start=(tap == 0), stop=(tap == 24),
                    )
                y2e = pp.tile([NK, 512], BF16, tag=f"y2e{ro}")
                if with_ebias:
                    nc.vector.scalar_tensor_tensor(
                        y2e[:], ps[:], 1.0, ebias[ro][:],
                        op0=mybir.AluOpType.mult, op1=mybir.AluOpType.add,
                    )
                else:
                    nc.vector.tensor_copy(y2e[:], ps[:])
                nc.scalar.activation(
                    y2e[:], y2e[:], mybir.ActivationFunctionType.Exp
                )
                return y2e

            # tap-sum via select matmul [100,4] -> [4, 512], then reciprocal
            def tapsums(y2e, ro):
                ps = psS.tile([4, 512], F32, tag="sums")
                nc.tensor.matmul(ps[:], wpk[0:NK, 64:68],
                                 y2e[:], start=True, stop=True)
                rs4 = pp.tile([4, 512], BF16, tag=f"rs4{ro}")
                with nc.allow_low_precision(
                    reason="softmax 1/sum in bf16; rel-tol is 2e-2"
                ):
                    nc.vector.reciprocal(rs4[:], ps[:])
                return rs4

            # normalize (broadcast 1/sum over partitions via selt matmul,
            # one DVE multiply), then regroup to taps-on-partitions with
            # sub on the free axis via 4 select matmuls + copies:
            #   yMs[kk, 128*w + 32*sub + tb] = y2e[4*kk+sub, f] * rs4[sub, f]
            def regroup(y2e, rs4, ro, yms):
                pB = psB.tile([NK, 512], F32, tag="bc")
                nc.tensor.matmul(
                    pB[:], wpk[0:4, 168:268], rs4[:], start=True, stop=True,
                )
                ymn = pp.tile([NK, 512], BF16, name=f"ymn{ro}", tag=f"ymn{ro}")
                nc.vector.tensor_tensor(
                    ymn[:], y2e[:], pB[:], op=mybir.AluOpType.mult,
                )
                for sub in range(4):
                    p25 = psRG.tile([25, 512], F32, tag="rg")
                    nc.tensor.matmul(
                        p25[:],
                        wpk[0:NK, 68 + sub * 25:68 + (sub + 1) * 25],
                        ymn[:], start=True, stop=True,
                    )
                    dst = AP(yms.tensor, sub * 32,
                             [[YMSF, 25], [128, 16], [1, 32]])
                    src = AP(p25.tensor, 0, [[512, 25], [32, 16], [1, 32]])
                    if sub % 2 == 0:
                        nc.vector.tensor_copy(dst, src)
                    else:
                        nc.scalar.copy(dst, src)

            # band scatter: per (ro, dii) one DMA into the DRAM scratch;
            # the w-diagonal (dst partition q = (ro+dii)*20 + w + djj AND
            # dst col 128*(16*ro + w) + ...) is a plain stride 4224 in flat
            # DRAM. Structural zeros come from the pre-zeroed output buf.
            def scatter(yms, ro):
                for dii in range(5):
                    src = AP(yms.tensor, dii * 5 * YMSF,
                             [[YMSF, 5], [128, 16], [1, 128]])
                    dst = AP(ydram_d,
                             (ro + dii) * 20 * YF + ro * 2048,
                             [[YF, 5], [YF + 128, 16], [1, 128]])
                    eng = nc.sync if dii % 2 == 0 else nc.scalar
                    eng.dma_start(dst, src)

            yms0 = pp.tile([25, YMSF], BF16, tag="yms0")
            yms1 = pp.tile([25, YMSF], BF16, tag="yms1")

            y2e0 = encode(0)
            rs40 = tapsums(y2e0, 0)
            y2e1 = encode(1)          # PE busy while DVE does recip ro0
            regroup(y2e0, rs40, 0, yms0)
            scatter(yms0, 0)
            rs41 = tapsums(y2e1, 1)
            regroup(y2e1, rs41, 1, yms1)
            scatter(yms1, 1)

            # ---- band load back + 25-tap MAC ----
            ybig = pp.tile([KDIM, YF], BF16, tag="ybig")
            nc.sync.dma_start(
                AP(ybig.tensor, 0, [[YF, KDIM], [1, YF]]),
                AP(ydram_d, 0, [[YF, KDIM], [1, YF]]),
            )

            osbs = [pp.tile([128, 2048], BF16, name=f"osb{i}", tag=f"osb{i}")
                    for i in range(4)]
            for blk in range(32):
                for ct in range(2):
                    ps = psMAC.tile([128, 128], F32, tag="mac")
                    nc.tensor.matmul(
                        ps[:],
                        AP(xcall.tensor, blk * 256 + ct * 128,
                           [[XCF, KDIM], [1, 128]]),
                        AP(ybig.tensor, blk, [[YF, KDIM], [32, 128]]),
                        start=True, stop=True,
                    )
                    osb = osbs[ct * 2 + blk // 16]
                    col = (blk % 16) * 128
                    if ct == 0:
                        nc.vector.tensor_copy(osb[:, col:col + 128], ps[:])
                    else:
                        nc.scalar.copy(osb[:, col:col + 128], ps[:])
                if blk == 15 or blk == 31:
                    half = blk // 16
                    for ct in range(2):
                        eng = nc.sync if ct == 0 else nc.scalar
                        eng.dma_start(
                            out_d[ct, :, half * 2048:(half + 1) * 2048],
                            osbs[ct * 2 + half][:],
                        )
    nc.compile()
    return nc


_CACHE: dict[bool, object] = {}


def _get_program(with_ebias: bool):
    if with_ebias not in _CACHE:
        _CACHE[with_ebias] = build_program(with_ebias)
    return _CACHE[with_ebias]


def _prep_inputs(x, w_comp, b_comp, w_enc, b_enc):
    """Build the per-core numpy input dicts."""
    import ml_dtypes

    bf16 = ml_dtypes.bfloat16
    x = np.asarray(x, dtype=np.float32)
    w_comp = np.asarray(w_comp, dtype=np.float32)
    b_comp = np.asarray(b_comp, dtype=np.float32)
    w_enc = np.asarray(w_enc, dtype=np.float32)
    b_enc = np.asarray(b_enc, dtype=np.float32)

    # packed weights: wcat | sel | Esel | esub
    wpk = np.zeros((128, WPKW), dtype=np.float32)
    for ct in range(2):
        wpk[:, ct * 32:(ct + 1) * 32] = w_comp[:, ct * 128:(ct + 1) * 128].T
    p = np.arange(NK)
    wpk[p, 64 + p % 4] = 1.0                       # sel
    wpk[p, 68 + (p % 4) * 25 + p // 4] = 1.0       # Esel
    wpk[p % 4, 168 + p] = 1.0                      # selt

    we = w_enc.reshape(NK, C_MID, 25)              # [o, m, tap]
    wet32 = np.ascontiguousarray(
        np.transpose(we, (1, 2, 0)).reshape(C_MID, 25 * NK)
    )

    with_ebias = bool(b_comp.any() or b_enc.any())

    in_maps = []
    for core in range(NCORES):
        b = core // 4
        h0 = (core % 4) * HSLICE
        xpad = np.zeros((C, ROWS, WP), dtype=np.float32)
        r_lo = max(0, h0 - 2)
        r_hi = min(H, h0 + HSLICE + 2)
        xpad[:, (r_lo - (h0 - 2)):(r_hi - (h0 - 2)), 2:2 + W] = x[b, :, r_lo:r_hi, :]

        xt2 = np.empty((ROWS, 20, 4, C), dtype=bf16)
        for b4 in range(4):
            xt2[:, :, b4, :] = xpad[:, :, b4 * 16:b4 * 16 + 20].transpose(1, 2, 0)

        m = {
            "xs": np.ascontiguousarray(xpad.reshape(2, 128, PADPOS)).astype(bf16),
            "xt2": xt2,
            "wpk": wpk.astype(bf16),
            "wet32": wet32.astype(bf16),
        }
        if with_ebias:
            # field[o, h, w] = b_enc[o] + sum over valid taps of w_enc.b_comp
            wb = np.einsum("omt,m->ot", we, b_comp).reshape(NK, 5, 5)
            field = np.zeros((NK, HSLICE, W), dtype=np.float32)
            for di in range(-2, 3):
                for dj in range(-2, 3):
                    hh = np.arange(h0, h0 + HSLICE)[:, None] + di
                    ww = np.arange(W)[None, :] + dj
                    valid = ((hh >= 0) & (hh < H) & (ww >= 0) & (ww < W))
                    field += (
                        wb[:, di + 2, dj + 2][:, None, None]
                        * valid[None].astype(np.float32)
                    )
            field += b_enc[:, None, None]
            f = field.reshape(NK, 8, 2, 4, 16)        # (o, tile, ro, b4, w)
            f = np.transpose(f, (2, 0, 4, 1, 3))      # (ro, o, w, tile, b4)
            m["ebias"] = np.ascontiguousarray(f.reshape(2, NK, 512))
        in_maps.append(m)
    return in_maps, with_ebias


TRACE = False
LAST_RESULT = None


def kernel(x, w_comp, b_comp, w_enc, b_enc):
    global LAST_RESULT
    from concourse.bass_utils import run_bass_kernel_spmd

    in_maps, with_ebias = _prep_inputs(x, w_comp, b_comp, w_enc, b_enc)
    nc = _get_program(with_ebias)
    res = run_bass_kernel_spmd(
        nc, in_maps, core_ids=list(range(NCORES)), trace=TRACE
    )
    LAST_RESULT = res
    out = np.empty((B, C, 2 * H, 2 * W), dtype=np.float32)
    for core in range(NCORES):
        b = core // 4
        h0 = (core % 4) * HSLICE
        o = np.asarray(res.results[core]["out"], dtype=np.float32)
        o = o.reshape(2, 128, 8, 4, 2, 16, 2, 2)
        # axes: (ct, c, g, b4, ro, w, r1, r2) -> (ct, c, g, ro, r1, b4, w, r2)
        o = np.transpose(o, (0, 1, 2, 4, 6, 3, 5, 7)).reshape(2, 128, 32, 128)
        out[b, :128, 2 * h0:2 * h0 + 32, :] = o[0]
        out[b, 128:, 2 * h0:2 * h0 + 32, :] = o[1]
    return out


# revision 24
# speedup vs baseline: 3.9718x; 1.4484x over previous
"""CARAFE content-aware upsampling on 8 Trainium2 NeuronCores (Bass/Tile).

Problem: x[2,256,64,64], 1x1 compress conv (256->32), 5x5 encoder conv
(32->100), pixel-shuffle(r=2) + softmax over 25 taps, then dynamic-filter
reassembly: out[b,c,2h+r1,2w+r2] = sum_k x[b,c,h+di,w+dj] * softmax_w.

Sharding: pure data-parallel over (batch, 16-row H slices) -> 8 cores.

Per-core pipeline (DMA-dispatch-minimal redesign):
  - compress (1x1) and encoder (5x5 as 25 PSUM-accumulated matmuls over
    shifted y1 views) run on PE in float32r (1 cyc/row at free>=256).
  - softmax stays channel-major; a per-sub select matmul then regroups
    the 100 = (25 taps x 4 subpixels) partition layout to taps-only
    partitions with sub on the free axis, folding in the 1/sum
    normalization (PE broadcast matmul + one DVE multiply per sub).
  - the banded 25-tap MAC operand is built via a DRAM round trip: the
    diagonal (partition+free coupled) strides live entirely on the DRAM
    side, so the whole scatter is 10 large DMAs into a pre-zeroed DRAM
    scratch + 1 dense load back (vs 160 per-column SBUF scatters).
  - x windows arrive pre-transposed from the host (xt2, bf16) and are
    gathered into the [120, g*b4*c] MAC stationary by 6 SWDGE DMAs.
  - the MAC runs in bf16 (1 cyc/row): 64 matmuls [120]x[128], psum
    [128c, 128n], results stored bf16 and upcast on the host.
"""

import sys

sys.path.insert(0, "/opt/trn_rl_repo")

import numpy as np

import concourse.bacc as bacc
import concourse.bass as bass
import concourse.tile as tile
from concourse import mybir
from concourse.ap import AP

F32 = mybir.dt.float32
F32R = mybir.dt.float32r
BF16 = mybir.dt.bfloat16

# geometry
B, C, H, W = 2, 256, 64, 64
RATIO, K_UP, C_MID, ENC_K = 2, 5, 32, 5
NK = RATIO * RATIO * K_UP * K_UP  # 100
HSLICE = 16                       # output source rows per core
ROWS = HSLICE + 4                 # with 2-row halo each side
WP = W + 4                        # padded width
PADPOS = ROWS * WP                # 1360
NCORES = 8

KDIM = 120                        # 6 rows x 20 cols window pixels
YF = 4096                         # ybig free dim: col = 32*n + blk
XCF = 8192                        # xcall free dim: (g, b4, c)
YMSF = 2048                       # yMs free dim: (w, sub, tb)
WPKW = 296                        # wpk cols: wcat 64 | sel 4 | Eall 128 | selt 100


def build_program(with_ebias: bool):
    nc = bacc.Bacc()
    xs_d = nc.declare_dram_parameter("xs", [2, 128, PADPOS], BF16, isOutput=False)
    xt2_d = nc.declare_dram_parameter("xt2", [ROWS, 20, 4, C], BF16, isOutput=False)
    wpk_d = nc.declare_dram_parameter("wpk", [128, WPKW], BF16, isOutput=False)
    wet_d = nc.declare_dram_parameter("wet32", [C_MID, 25 * NK], BF16, isOutput=False)
    wetq_d = nc.declare_dram_parameter("wetq", [128, 600], BF16, isOutput=False)
    if with_ebias:
        ebias_d = nc.declare_dram_parameter("ebias", [2, NK, 512], F32, isOutput=False)
    # band scratch; ExternalOutput buffers are pre-zeroed by the runtime,
    # which supplies the structural zeros between the band diagonals.
    ydram_d = nc.declare_dram_parameter("ydram", [KDIM, YF], BF16, isOutput=True)
    out_d = nc.declare_dram_parameter("out", [128, 8192], BF16, isOutput=True)

    with tile.TileContext(nc) as tc:
        # The diagonal DRAM scatter APs (partition+free coupled strides on
        # the DRAM side) confuse the byte-range race detector; dependency
        # generation itself is tensor-granular and conservative.
        tc.race_detector_enabled = False
        with (
            tc.tile_pool(name="persist", bufs=1) as pp,
            tc.tile_pool(name="psCMP", bufs=1, space="PSUM") as psCMP,
            tc.tile_pool(name="psENC", bufs=1, space="PSUM") as psENC,
            tc.tile_pool(name="psS", bufs=1, space="PSUM") as psS,
            tc.tile_pool(name="psRG", bufs=1, space="PSUM") as psRG,
            tc.tile_pool(name="psB", bufs=1, space="PSUM") as psB,
            tc.tile_pool(name="psMAC", bufs=3, space="PSUM") as psMAC,
        ):
            # ---- input loads (split so compress can start early) ----
            xst = []
            for ct in range(2):
                t = pp.tile([128, PADPOS], BF16, name=f"xst{ct}", tag=f"xs{ct}")
                xst.append(t)
            for piece in ((0, 512), (512, 1024), (1024, PADPOS)):
                for ct in range(2):
                    nc.sync.dma_start(
                        xst[ct][:, piece[0]:piece[1]],
                        xs_d[ct, :, piece[0]:piece[1]],
                    )

            wpk = pp.tile([128, WPKW], BF16, tag="wpk")
            nc.scalar.dma_start(wpk[:], wpk_d[:])
            wetb = pp.tile([C_MID, 25 * NK], BF16, tag="wetb")
            nc.scalar.dma_start(wetb[:], wet_d[:])
            wetq = pp.tile([128, 600], BF16, tag="wetq")
            nc.scalar.dma_start(wetq[:], wetq_d[:])

            if with_ebias:
                ebias = []
                for ro in range(2):
                    t = pp.tile([NK, 512], F32, tag=f"ebias{ro}")
                    nc.scalar.dma_start(t[:], ebias_d[ro])
                    ebias.append(t)

            # xcall[rq*20+cq, (g, b4, c)] = x[c, 2g+rq, b4*16+cq] (padded
            # coords), gathered from the host-transposed xt2 on the SWDGE
            # queue (Pool engine) to keep HWDGE free.
            xcall = pp.tile([KDIM, XCF], BF16, tag="xcall")
            for rq in range(6):
                dst = AP(xcall.tensor, rq * 20 * XCF,
                         [[XCF, 20], [1024, 8], [1, 1024]])
                src = AP(xt2_d, rq * 20480,
                         [[1024, 20], [40960, 8], [1, 1024]])
                nc.gpsimd.dma_start(dst, src)

            # ---- compress conv y1[32, PADPOS] (f32r) ----
            y1 = pp.tile([C_MID, PADPOS], BF16, tag="y1")
            off = 0
            ci = 0
            while off < PADPOS:
                n = min(512, PADPOS - off)
                ps = psCMP.tile([C_MID, 512], F32, tag="cmp")
                for ct in range(2):
                    nc.tensor.matmul(
                        ps[:, :n],
                        wpk[:, ct * 32:(ct + 1) * 32],
                        xst[ct][:, off:off + n],
                        start=(ct == 0), stop=(ct == 1),
                    )
                eng = nc.vector if ci % 2 == 0 else nc.scalar
                if eng is nc.vector:
                    eng.tensor_copy(y1[:, off:off + n], ps[:, :n])
                else:
                    eng.copy(y1[:, off:off + n], ps[:, :n])
                off += n
                ci += 1

            # ---- shifted y1 stack: 4 taps per encoder matmul ----
            # y1stack[tl*32+m, p] = y1[m, p + s_tl], s_tl in {0, WP, 1, WP+1}
            # so a 2x2 tap block (dii0+a, djj0+b) contracts 128 partitions.
            STK = PADPOS - WP - 1
            y1stack = pp.tile([128, PADPOS], BF16, tag="y1stack")
            for tl, s in enumerate((0, WP, 1, WP + 1)):
                eng = nc.vector if tl % 2 == 0 else nc.scalar
                dst = y1stack[tl * 32:(tl + 1) * 32, 0:STK]
                src = y1[:, s:s + STK]
                if tl % 2 == 0:
                    eng.tensor_copy(dst, src)
                else:
                    eng.copy(dst, src)

            # ---- encoder conv + exp, per output-row parity ro ----
            # rhs columns stream in pos' = (w, tile, b4) order:
            # f = 32*w + 4*tile + b4  <->  (h = 2*tile + ro, wcol = 16*b4 + w)
            # 11 matmuls: 5 singles (dii=4, from y1), 2 pairs (djj=4, from
            # the {0,WP} stack rows), 4 quads (2x2 tap blocks, full stack).
            def encode(ro):
                ps = psENC.tile([NK, 512], F32, tag="enc")
                POSDIMS = [[1, 16], [2 * WP, 8], [16, 4]]
                groups = []
                for djj in range(5):    # singles: tap (dii=4, djj)
                    groups.append((y1.tensor, C_MID,
                                   (ro + 4) * WP + djj,
                                   wetb[:, (20 + djj) * NK:(21 + djj) * NK]))
                for gi, dii0 in enumerate((0, 2)):   # pairs: (dii0+a, djj=4)
                    groups.append((y1stack.tensor, 64,
                                   (ro + dii0) * WP + 4,
                                   wetq[0:64, 400 + gi * 100:500 + gi * 100]))
                qi = 0
                for dii0 in (0, 2):                  # quads
                    for djj0 in (0, 2):
                        groups.append((y1stack.tensor, 128,
                                       (ro + dii0) * WP + djj0,
                                       wetq[:, qi * 100:(qi + 1) * 100]))
                        qi += 1
                for i, (mt, cp, moff, statw) in enumerate(groups):
                    rhs = AP(mt, moff, [[PADPOS, cp]] + POSDIMS)
                    nc.tensor.matmul(
                        ps[:], statw, rhs,
                        start=(i == 0), stop=(i == len(groups) - 1),
                    )
                y2e = pp.tile([NK, 512], BF16, tag=f"y2e{ro}")
                if with_ebias:
                    nc.vector.scalar_tensor_tensor(
                        y2e[:], ps[:], 1.0, ebias[ro][:],
                        op0=mybir.AluOpType.mult, op1=mybir.AluOpType.add,
                    )
                    nc.scalar.activation(
                        y2e[:], y2e[:], mybir.ActivationFunctionType.Exp
                    )
                else:
                    nc.scalar.activation(
                        y2e[:], ps[:], mybir.ActivationFunctionType.Exp
                    )
                return y2e

            # tap-sum via select matmul [100,4] -> [4, 512], then reciprocal
            def tapsums(y2e, ro):
                ps = psS.tile([4, 512], F32, tag="sums")
                nc.tensor.matmul(ps[:], wpk[0:NK, 64:68],
                                 y2e[:], start=True, stop=True)
                rs4 = pp.tile([4, 512], BF16, tag=f"rs4{ro}")
                with nc.allow_low_precision(
                    reason="softmax 1/sum in bf16; rel-tol is 2e-2"
                ):
                    nc.vector.reciprocal(rs4[:], ps[:])
                return rs4

            # normalize (broadcast 1/sum over partitions via selt matmul,
            # one DVE multiply), then regroup to taps-on-partitions with
            # sub on the free axis via 4 select matmuls + copies:
            #   yMs[kk, 128*w + 32*sub + tb] = y2e[4*kk+sub, f] * rs4[sub, f]
            def regroup(y2e, rs4, ro, yms):
                pB = psB.tile([NK, 512], F32, tag="bc")
                nc.tensor.matmul(
                    pB[:], wpk[0:4, 196:296], rs4[:], start=True, stop=True,
                )
                ymn = pp.tile([NK, 512], BF16, name=f"ymn{ro}", tag=f"ymn{ro}")
                nc.vector.tensor_tensor(
                    ymn[:], y2e[:], pB[:], op=mybir.AluOpType.mult,
                )
                # one permutation matmul [100,100], then 4 psum-slice copies
                pG = psRG.tile([128, 512], F32, tag="rg")
                nc.tensor.matmul(
                    pG[:], wpk[0:NK, 68:196], ymn[:], start=True, stop=True,
                )
                for sub in range(4):
                    dst = AP(yms.tensor, sub * 32,
                             [[YMSF, 25], [128, 16], [1, 32]])
                    src = AP(pG.tensor, sub * 32 * 512,
                             [[512, 25], [32, 16], [1, 32]])
                    if sub % 2 == 0:
                        nc.vector.tensor_copy(dst, src)
                    else:
                        nc.scalar.copy(dst, src)

            # band scatter: per (ro, dii) one DMA into the DRAM scratch;
            # the w-diagonal (dst partition q = (ro+dii)*20 + w + djj AND
            # dst col 128*(16*ro + w) + ...) is a plain stride 4224 in flat
            # DRAM. Structural zeros come from the pre-zeroed output buf.
            def scatter(yms, ro):
                for dii in range(5):
                    src = AP(yms.tensor, dii * 5 * YMSF,
                             [[YMSF, 5], [128, 16], [1, 128]])
                    dst = AP(ydram_d,
                             (ro + dii) * 20 * YF + ro * 2048,
                             [[YF, 5], [YF + 128, 16], [1, 128]])
                    eng = (nc.sync, nc.scalar, nc.gpsimd, nc.sync,
                           nc.scalar)[dii]
                    eng.dma_start(dst, src)

            yms0 = pp.tile([25, YMSF], BF16, tag="yms0")
            yms1 = pp.tile([25, YMSF], BF16, tag="yms1")

            y2e0 = encode(0)
            rs40 = tapsums(y2e0, 0)
            y2e1 = encode(1)          # PE busy while DVE does recip ro0
            regroup(y2e0, rs40, 0, yms0)
            scatter(yms0, 0)
            rs41 = tapsums(y2e1, 1)
            regroup(y2e1, rs41, 1, yms1)
            scatter(yms1, 1)

            # ---- band load back (row-chunked) + 25-tap MAC ----
            # psum [n, c]: stationary = ybig block (strided cols), moving =
            # xcall full-channel slice (256 rows/matmul, 1 mm per block).
            ybig = pp.tile([KDIM, YF], BF16, tag="ybig")
            for i, (r0, r1) in enumerate(((0, 40), (40, 80), (80, KDIM))):
                eng = nc.sync if i % 2 == 0 else nc.scalar
                eng.dma_start(
                    AP(ybig.tensor, r0 * YF, [[YF, r1 - r0], [1, YF]]),
                    AP(ydram_d, r0 * YF, [[YF, r1 - r0], [1, YF]]),
                )

            osbs = [pp.tile([128, 2048], BF16, name=f"osb{i}", tag=f"osb{i}")
                    for i in range(4)]
            for k in range(16):             # 2 blocks per psum tile
                ps = psMAC.tile([128, 512], F32, tag="mac")
                for j in range(2):
                    blk = k * 2 + j
                    nc.tensor.matmul(
                        ps[:, j * 256:(j + 1) * 256],
                        AP(ybig.tensor, blk, [[YF, KDIM], [32, 128]]),
                        AP(xcall.tensor, blk * 256, [[XCF, KDIM], [1, 256]]),
                        start=True, stop=True,
                    )
                osb = osbs[k // 4]
                col = (k % 4) * 512
                if k % 2 == 0:
                    nc.vector.tensor_copy(osb[:, col:col + 512], ps[:])
                else:
                    nc.scalar.copy(osb[:, col:col + 512], ps[:])
                if k % 4 == 3:
                    q = k // 4
                    eng = nc.sync if q % 2 == 0 else nc.scalar
                    eng.dma_start(
                        out_d[:, q * 2048:(q + 1) * 2048], osbs[q][:],
                    )
    nc.compile()
    return nc


_CACHE: dict[bool, object] = {}


def _get_program(with_ebias: bool):
    if with_ebias not in _CACHE:
        _CACHE[with_ebias] = build_program(with_ebias)
    return _CACHE[with_ebias]


def _prep_inputs(x, w_comp, b_comp, w_enc, b_enc):
    """Build the per-core numpy input dicts."""
    import ml_dtypes

    bf16 = ml_dtypes.bfloat16
    x = np.asarray(x, dtype=np.float32)
    w_comp = np.asarray(w_comp, dtype=np.float32)
    b_comp = np.asarray(b_comp, dtype=np.float32)
    w_enc = np.asarray(w_enc, dtype=np.float32)
    b_enc = np.asarray(b_enc, dtype=np.float32)

    # packed weights: wcat | sel | Esel | esub
    wpk = np.zeros((128, WPKW), dtype=np.float32)
    for ct in range(2):
        wpk[:, ct * 32:(ct + 1) * 32] = w_comp[:, ct * 128:(ct + 1) * 128].T
    p = np.arange(NK)
    wpk[p, 64 + p % 4] = 1.0                       # sel
    wpk[p, 68 + (p % 4) * 32 + p // 4] = 1.0       # Eall (32-aligned subs)
    wpk[p % 4, 196 + p] = 1.0                      # selt

    we = w_enc.reshape(NK, C_MID, 25)              # [o, m, tap]
    wet32 = np.ascontiguousarray(
        np.transpose(we, (1, 2, 0)).reshape(C_MID, 25 * NK)
    )

    # wetq: stacked-encoder stationaries. Stack row tl = b*2 + a carries
    # shift a*WP + b, so quad (dii0, djj0) row (tl, m) = w_enc tap
    # (dii0+a, djj0+b); pairs cover (dii0+a, djj=4) on rows tl in {0,1}.
    wetq = np.zeros((128, 600), dtype=np.float32)
    wet_om = we.reshape(NK, C_MID, 5, 5)           # [o, m, dii, djj]
    qi = 0
    for dii0 in (0, 2):
        for djj0 in (0, 2):
            for tl in range(4):
                a, b = tl % 2, tl // 2
                wetq[tl * 32:(tl + 1) * 32, qi * 100:(qi + 1) * 100] = \
                    wet_om[:, :, dii0 + a, djj0 + b].T
            qi += 1
    for gi, dii0 in enumerate((0, 2)):
        for a in range(2):
            wetq[a * 32:(a + 1) * 32, 400 + gi * 100:500 + gi * 100] = \
                wet_om[:, :, dii0 + a, 4].T

    with_ebias = bool(b_comp.any() or b_enc.any())

    in_maps = []
    for core in range(NCORES):
        b = core // 4
        h0 = (core % 4) * HSLICE
        xpad = np.zeros((C, ROWS, WP), dtype=np.float32)
        r_lo = max(0, h0 - 2)
        r_hi = min(H, h0 + HSLICE + 2)
        xpad[:, (r_lo - (h0 - 2)):(r_hi - (h0 - 2)), 2:2 + W] = x[b, :, r_lo:r_hi, :]

        xt2 = np.empty((ROWS, 20, 4, C), dtype=bf16)
        for b4 in range(4):
            xt2[:, :, b4, :] = xpad[:, :, b4 * 16:b4 * 16 + 20].transpose(1, 2, 0)

        m = {
            "xs": np.ascontiguousarray(xpad.reshape(2, 128, PADPOS)).astype(bf16),
            "xt2": xt2,
            "wpk": wpk.astype(bf16),
            "wet32": wet32.astype(bf16),
            "wetq": wetq.astype(bf16),
        }
        if with_ebias:
            # field[o, h, w] = b_enc[o] + sum over valid taps of w_enc.b_comp
            wb = np.einsum("omt,m->ot", we, b_comp).reshape(NK, 5, 5)
            field = np.zeros((NK, HSLICE, W), dtype=np.float32)
            for di in range(-2, 3):
                for dj in range(-2, 3):
                    hh = np.arange(h0, h0 + HSLICE)[:, None] + di
                    ww = np.arange(W)[None, :] + dj
                    valid = ((hh >= 0) & (hh < H) & (ww >= 0) & (ww < W))
                    field += (
                        wb[:, di + 2, dj + 2][:, None, None]
                        * valid[None].astype(np.float32)
                    )
            field += b_enc[:, None, None]
            f = field.reshape(NK, 8, 2, 4, 16)        # (o, tile, ro, b4, w)
            f = np.transpose(f, (2, 0, 4, 1, 3))      # (ro, o, w, tile, b4)
            m["ebias"] = np.ascontiguousarray(f.reshape(2, NK, 512))
        in_maps.append(m)
    return in_maps, with_ebias


TRACE = False
LAST_RESULT = None


def kernel(x, w_comp, b_comp, w_enc, b_enc):
    global LAST_RESULT
    from concourse.bass_utils import run_bass_kernel_spmd

    in_maps, with_ebias = _prep_inputs(x, w_comp, b_comp, w_enc, b_enc)
    nc = _get_program(with_ebias)
    res = run_bass_kernel_spmd(
        nc, in_maps, core_ids=list(range(NCORES)), trace=TRACE
    )
    LAST_RESULT = res
    out = np.empty((B, C, 2 * H, 2 * W), dtype=np.float32)
    for core in range(NCORES):
        b = core // 4
        h0 = (core % 4) * HSLICE
        o = np.asarray(res.results[core]["out"], dtype=np.float32)
        # rows n = (ro, w, r1, r2), cols = (g, b4, c)
        o = o.reshape(2, 16, 2, 2, 8, 4, 256)
        # -> (c, g, ro, r1, b4, w, r2)
        o = np.transpose(o, (6, 4, 0, 2, 5, 1, 3)).reshape(256, 32, 128)
        out[b, :, 2 * h0:2 * h0 + 32, :] = o
    return out


# revision 37
# speedup vs baseline: 4.4474x; 1.1197x over previous
"""CARAFE content-aware upsampling on 8 Trainium2 NeuronCores (Bass/Tile).

Problem: x[2,256,64,64], 1x1 compress conv (256->32), 5x5 encoder conv
(32->100), pixel-shuffle(r=2) + softmax over 25 taps, then dynamic-filter
reassembly: out[b,c,2h+r1,2w+r2] = sum_k x[b,c,h+di,w+dj] * softmax_w.

Sharding: pure data-parallel over (batch, 16-row H slices) -> 8 cores.

Per-core pipeline (DMA-dispatch-minimal redesign, all-bf16 matmuls at
1 cyc/row; rel-tol is 2e-2 so bf16 error ~0.5% is fine):
  - compress (1x1) on PE; a 4-way shifted copy of y1 on 128 partitions
    (y1stack) lets the encoder contract 2x2 tap blocks, so the 5x5 conv
    is 11 matmuls per row-parity instead of 25.
  - softmax stays channel-major; a [100,128] permutation matmul then
    regroups (25 taps x 4 subpixels) partitions to taps-only partitions
    with sub on the free axis (32-aligned PSUM slices), after a
    broadcast matmul + one DVE multiply fold in the 1/sum.
  - the banded 25-tap MAC operand is built via a DRAM round trip: the
    diagonal (partition+free coupled) strides live entirely on the DRAM
    side, so the whole scatter is 10 large DMAs into a pre-zeroed DRAM
    scratch (ExternalOutput buffers arrive zeroed = the band gaps) + 6
    row-chunk loads back, instead of 160 per-column SBUF scatters.
  - x windows arrive pre-transposed from the host (xt2, bf16) and are
    gathered into the [120, g*b4*c] xcall operand by 6 SWDGE DMAs.
  - MAC: 32 matmuls (stationary = band block, moving = xcall 256-chan
    slice), psum [128n, 256c] pairs, bf16 results upcast on the host.
"""

import sys

sys.path.insert(0, "/opt/trn_rl_repo")

import numpy as np

import concourse.bacc as bacc
import concourse.bass as bass
import concourse.tile as tile
from concourse import mybir
from concourse.ap import AP

F32 = mybir.dt.float32
F32R = mybir.dt.float32r
BF16 = mybir.dt.bfloat16

# geometry
B, C, H, W = 2, 256, 64, 64
RATIO, K_UP, C_MID, ENC_K = 2, 5, 32, 5
NK = RATIO * RATIO * K_UP * K_UP  # 100
HSLICE = 16                       # output source rows per core
ROWS = HSLICE + 4                 # with 2-row halo each side
WP = W + 4                        # padded width
PADPOS = ROWS * WP                # 1360
NCORES = 8

KDIM = 120                        # 6 rows x 20 cols window pixels
YF = 4096                         # ybig free dim: col = 32*n + blk
XCF = 8192                        # xcall free dim: (g, b4, c)
YMSF = 2048                       # yMs free dim: (w, sub, tb)
WPKW = 296                        # wpk cols: wcat 64 | sel 4 | Eall 128 | selt 100


def build_program(with_ebias: bool):
    nc = bacc.Bacc()
    xs_d = nc.declare_dram_parameter("xs", [2, 128, PADPOS], BF16, isOutput=False)
    xt2_d = nc.declare_dram_parameter("xt2", [ROWS, 20, 4, C], BF16, isOutput=False)
    wpk_d = nc.declare_dram_parameter("wpk", [128, WPKW], BF16, isOutput=False)
    wet_d = nc.declare_dram_parameter("wet32", [C_MID, 25 * NK], BF16, isOutput=False)
    wetq_d = nc.declare_dram_parameter("wetq", [128, 600], BF16, isOutput=False)
    if with_ebias:
        ebias_d = nc.declare_dram_parameter("ebias", [2, NK, 512], F32, isOutput=False)
    # band scratch; ExternalOutput buffers are pre-zeroed by the runtime,
    # which supplies the structural zeros between the band diagonals.
    ydram_d = nc.declare_dram_parameter("ydram", [KDIM, YF], BF16, isOutput=True)
    out_d = nc.declare_dram_parameter("out", [128, 8192], BF16, isOutput=True)

    with tile.TileContext(nc) as tc:
        # The diagonal DRAM scatter APs (partition+free coupled strides on
        # the DRAM side) confuse the byte-range race detector; dependency
        # generation itself is tensor-granular and conservative.
        tc.race_detector_enabled = False
        with tc.tile_pool(name="persist", bufs=1) as pp:
            # ---- input loads (split so compress can start early) ----
            wpk = pp.tile([128, WPKW], BF16, tag="wpk")
            nc.scalar.dma_start(wpk[:], wpk_d[:])

            xst = []
            for ct in range(2):
                t = pp.tile([128, PADPOS], BF16, name=f"xst{ct}", tag=f"xs{ct}")
                xst.append(t)
            for piece in ((0, 512), (512, 1024), (1024, PADPOS)):
                for ct in range(2):
                    nc.sync.dma_start(
                        xst[ct][:, piece[0]:piece[1]],
                        xs_d[ct, :, piece[0]:piece[1]],
                    )

            wetb = pp.tile([C_MID, 25 * NK], BF16, tag="wetb")
            nc.scalar.dma_start(wetb[:], wet_d[:])
            wetq = pp.tile([128, 600], BF16, tag="wetq")
            nc.scalar.dma_start(wetq[:], wetq_d[:])

            if with_ebias:
                ebias = []
                for ro in range(2):
                    t = pp.tile([NK, 512], F32, tag=f"ebias{ro}")
                    nc.scalar.dma_start(t[:], ebias_d[ro])
                    ebias.append(t)

            # xcall[rq*20+cq, (g, b4, c)] = x[c, 2g+rq, b4*16+cq] (padded
            # coords), gathered from the host-transposed xt2 on the SWDGE
            # queue (Pool engine) to keep HWDGE free.
            xcall = pp.tile([KDIM, XCF], BF16, tag="xcall")
            for rq in range(6):
                dst = AP(xcall.tensor, rq * 20 * XCF,
                         [[XCF, 20], [1024, 8], [1, 1024]])
                src = AP(xt2_d, rq * 20480,
                         [[1024, 20], [40960, 8], [1, 1024]])
                nc.gpsimd.dma_start(dst, src)

            # ---- compress conv y1[32, PADPOS] ----
            psCMP = tc.alloc_tile_pool(name="psCMP", bufs=2, space="PSUM")
            y1 = pp.tile([C_MID, PADPOS], BF16, tag="y1")
            # shifted stack alongside y1: y1stack[tl*32+m, p] = y1[m, p+s_tl]
            # with s_tl in {0, WP, 1, WP+1}, so a 2x2 tap block contracts
            # 128 partitions. Built per compress chunk straight from PSUM.
            STK = PADPOS - WP - 1
            SHIFTS = (0, WP, 1, WP + 1)
            y1stack = pp.tile([128, PADPOS], BF16, tag="y1stack")
            off = 0
            ci = 0
            while off < PADPOS:
                n = min(512, PADPOS - off)
                ps = psCMP.tile([C_MID, 512], F32, tag="cmp")
                for ct in range(2):
                    nc.tensor.matmul(
                        ps[:, :n],
                        wpk[:, ct * 32:(ct + 1) * 32],
                        xst[ct][:, off:off + n],
                        start=(ct == 0), stop=(ct == 1),
                    )
                eng = nc.vector if ci % 2 == 0 else nc.scalar
                if eng is nc.vector:
                    eng.tensor_copy(y1[:, off:off + n], ps[:, :n])
                else:
                    eng.copy(y1[:, off:off + n], ps[:, :n])
                # stack slices fed by this chunk (read from y1 SBUF so the
                # psum tile is released immediately): stack cols
                # [off-s, off+n-s) <- y1 cols [off, off+n)
                for tl, sh in enumerate(SHIFTS):
                    a = max(0, off - sh)
                    b = min(STK, off + n - sh)
                    if b <= a:
                        continue
                    dst = y1stack[tl * 32:(tl + 1) * 32, a:b]
                    src = y1[:, a + sh:b + sh]
                    if tl % 3 == 0:
                        nc.gpsimd.tensor_copy(dst, src)
                    elif tl % 3 == 1:
                        nc.vector.tensor_copy(dst, src)
                    else:
                        nc.scalar.copy(dst, src)
                off += n
                ci += 1


            psCMP.release()
            psENC = tc.alloc_tile_pool(name="psENC", bufs=2, space="PSUM")
            psS = tc.alloc_tile_pool(name="psS", bufs=1, space="PSUM")
            psRG = tc.alloc_tile_pool(name="psRG", bufs=2, space="PSUM")
            psB = tc.alloc_tile_pool(name="psB", bufs=2, space="PSUM")

            # ---- encoder conv + exp, per output-row parity ro ----
            # rhs columns stream in pos' = (w, tile, b4) order:
            # f = 32*w + 4*tile + b4  <->  (h = 2*tile + ro, wcol = 16*b4 + w)
            # 11 matmuls: 5 singles (dii=4, from y1), 2 pairs (djj=4, from
            # the {0,WP} stack rows), 4 quads (2x2 tap blocks, full stack).
            def encode(ro):
                ps = psENC.tile([NK, 512], F32, tag="enc")
                POSDIMS = [[1, 16], [2 * WP, 8], [16, 4]]
                groups = []
                for djj in range(5):    # singles: tap (dii=4, djj)
                    groups.append((y1.tensor, C_MID,
                                   (ro + 4) * WP + djj,
                                   wetb[:, (20 + djj) * NK:(21 + djj) * NK]))
                for gi, dii0 in enumerate((0, 2)):   # pairs: (dii0+a, djj=4)
                    groups.append((y1stack.tensor, 64,
                                   (ro + dii0) * WP + 4,
                                   wetq[0:64, 400 + gi * 100:500 + gi * 100]))
                qi = 0
                for dii0 in (0, 2):                  # quads
                    for djj0 in (0, 2):
                        groups.append((y1stack.tensor, 128,
                                       (ro + dii0) * WP + djj0,
                                       wetq[:, qi * 100:(qi + 1) * 100]))
                        qi += 1
                for i, (mt, cp, moff, statw) in enumerate(groups):
                    rhs = AP(mt, moff, [[PADPOS, cp]] + POSDIMS)
                    nc.tensor.matmul(
                        ps[:], statw, rhs,
                        start=(i == 0), stop=(i == len(groups) - 1),
                    )
                y2e = pp.tile([NK, 512], BF16, tag=f"y2e{ro}")
                if with_ebias:
                    nc.vector.scalar_tensor_tensor(
                        y2e[:], ps[:], 1.0, ebias[ro][:],
                        op0=mybir.AluOpType.mult, op1=mybir.AluOpType.add,
                    )
                    nc.scalar.activation(
                        y2e[:], y2e[:], mybir.ActivationFunctionType.Exp
                    )
                else:
                    nc.scalar.activation(
                        y2e[:], ps[:], mybir.ActivationFunctionType.Exp
                    )
                return y2e

            # tap-sum via select matmul [100,4] -> [4, 512], then reciprocal
            def tapsums(y2e, ro):
                ps = psS.tile([4, 512], F32, tag="sums")
                nc.tensor.matmul(ps[:], wpk[0:NK, 64:68],
                                 y2e[:], start=True, stop=True)
                rs4 = pp.tile([4, 512], BF16, tag=f"rs4{ro}")
                with nc.allow_low_precision(
                    reason="softmax 1/sum in bf16; rel-tol is 2e-2"
                ):
                    nc.vector.reciprocal(rs4[:], ps[:])
                return rs4

            # normalize (broadcast 1/sum over partitions via selt matmul,
            # one DVE multiply), then regroup to taps-on-partitions with
            # sub on the free axis via 4 select matmuls + copies:
            #   yMs[kk, 128*w + 32*sub + tb] = y2e[4*kk+sub, f] * rs4[sub, f]
            def regroup(y2e, rs4, ro, yms):
                pB = psB.tile([NK, 512], F32, tag="bc")
                nc.tensor.matmul(
                    pB[:], wpk[0:4, 196:296], rs4[:], start=True, stop=True,
                )
                ymn = pp.tile([NK, 512], BF16, name=f"ymn{ro}", tag=f"ymn{ro}")
                nc.vector.tensor_tensor(
                    ymn[:], y2e[:], pB[:], op=mybir.AluOpType.mult,
                )
                # one permutation matmul [100,100], then 4 psum-slice copies
                pG = psRG.tile([128, 512], F32, tag="rg")
                nc.tensor.matmul(
                    pG[:], wpk[0:NK, 68:196], ymn[:], start=True, stop=True,
                )
                for sub in range(4):
                    dst = AP(yms.tensor, sub * 32,
                             [[YMSF, 25], [128, 16], [1, 32]])
                    src = AP(pG.tensor, sub * 32 * 512,
                             [[512, 25], [32, 16], [1, 32]])
                    if sub % 2 == 0:
                        nc.vector.tensor_copy(dst, src)
                    else:
                        nc.scalar.copy(dst, src)

            # band scatter: per (ro, dii) one DMA into the DRAM scratch;
            # the w-diagonal (dst partition q = (ro+dii)*20 + w + djj AND
            # dst col 128*(16*ro + w) + ...) is a plain stride 4224 in flat
            # DRAM. Structural zeros come from the pre-zeroed output buf.
            def scatter(yms, ro):
                for dii in range(5):
                    src = AP(yms.tensor, dii * 5 * YMSF,
                             [[YMSF, 5], [128, 16], [1, 128]])
                    dst = AP(ydram_d,
                             (ro + dii) * 20 * YF + ro * 2048,
                             [[YF, 5], [YF + 128, 16], [1, 128]])
                    eng = (nc.sync, nc.scalar, nc.gpsimd, nc.sync,
                           nc.scalar)[dii]
                    eng.dma_start(dst, src)

            yms0 = pp.tile([25, YMSF], BF16, tag="yms0")
            yms1 = pp.tile([25, YMSF], BF16, tag="yms1")

            y2e0 = encode(0)
            rs40 = tapsums(y2e0, 0)
            y2e1 = encode(1)          # PE busy while DVE does recip ro0
            regroup(y2e0, rs40, 0, yms0)
            scatter(yms0, 0)
            rs41 = tapsums(y2e1, 1)
            regroup(y2e1, rs41, 1, yms1)
            scatter(yms1, 1)

            for pool in (psB, psRG, psS, psENC):
                pool.release()
            psMAC = tc.alloc_tile_pool(name="psMAC", bufs=6, space="PSUM")

            # ---- band load back (row-chunked) + 25-tap MAC ----
            # psum [n, c]: stationary = ybig block (strided cols), moving =
            # xcall full-channel slice (256 rows/matmul, 1 mm per block).
            ybig = pp.tile([KDIM, YF], BF16, tag="ybig")
            for i in range(6):
                r0, r1 = i * 20, (i + 1) * 20
                eng = (nc.sync, nc.scalar, nc.gpsimd)[i % 3]
                eng.dma_start(
                    AP(ybig.tensor, r0 * YF, [[YF, r1 - r0], [1, YF]]),
                    AP(ydram_d, r0 * YF, [[YF, r1 - r0], [1, YF]]),
                )

            osbs = [pp.tile([128, 2048], BF16, name=f"osb{i}", tag=f"osb{i}")
                    for i in range(4)]
            for k in range(16):             # 2 blocks per psum tile
                ps = psMAC.tile([128, 512], F32, tag="mac")
                for j in range(2):
                    blk = k * 2 + j
                    nc.tensor.matmul(
                        ps[:, j * 256:(j + 1) * 256],
                        AP(ybig.tensor, blk, [[YF, KDIM], [32, 128]]),
                        AP(xcall.tensor, blk * 256, [[XCF, KDIM], [1, 256]]),
                        start=True, stop=True,
                    )
                osb = osbs[k // 4]
                col = (k % 4) * 512
                if k % 2 == 0:
                    nc.vector.tensor_copy(osb[:, col:col + 512], ps[:])
                else:
                    nc.scalar.copy(osb[:, col:col + 512], ps[:])
                if k % 2 == 1:
                    # ship each half-quarter as soon as its copies land
                    q, h = k // 4, (k % 4) // 2
                    eng = nc.sync if k % 4 == 1 else nc.scalar
                    eng.dma_start(
                        out_d[:, q * 2048 + h * 1024:q * 2048 + h * 1024 + 1024],
                        osbs[q][:, h * 1024:(h + 1) * 1024],
                    )
            psMAC.release()
    nc.compile()
    return nc


_CACHE: dict[bool, object] = {}


def _get_program(with_ebias: bool):
    if with_ebias not in _CACHE:
        _CACHE[with_ebias] = build_program(with_ebias)
    return _CACHE[with_ebias]


def _prep_inputs(x, w_comp, b_comp, w_enc, b_enc):
    """Build the per-core numpy input dicts."""
    import ml_dtypes

    bf16 = ml_dtypes.bfloat16
    x = np.asarray(x, dtype=np.float32)
    w_comp = np.asarray(w_comp, dtype=np.float32)
    b_comp = np.asarray(b_comp, dtype=np.float32)
    w_enc = np.asarray(w_enc, dtype=np.float32)
    b_enc = np.asarray(b_enc, dtype=np.float32)

    # packed weights: wcat | sel | Esel | esub
    wpk = np.zeros((128, WPKW), dtype=np.float32)
    for ct in range(2):
        wpk[:, ct * 32:(ct + 1) * 32] = w_comp[:, ct * 128:(ct + 1) * 128].T
    p = np.arange(NK)
    wpk[p, 64 + p % 4] = 1.0                       # sel
    wpk[p, 68 + (p % 4) * 32 + p // 4] = 1.0       # Eall (32-aligned subs)
    wpk[p % 4, 196 + p] = 1.0                      # selt

    we = w_enc.reshape(NK, C_MID, 25)              # [o, m, tap]
    wet32 = np.ascontiguousarray(
        np.transpose(we, (1, 2, 0)).reshape(C_MID, 25 * NK)
    )

    # wetq: stacked-encoder stationaries. Stack row tl = b*2 + a carries
    # shift a*WP + b, so quad (dii0, djj0) row (tl, m) = w_enc tap
    # (dii0+a, djj0+b); pairs cover (dii0+a, djj=4) on rows tl in {0,1}.
    wetq = np.zeros((128, 600), dtype=np.float32)
    wet_om = we.reshape(NK, C_MID, 5, 5)           # [o, m, dii, djj]
    qi = 0
    for dii0 in (0, 2):
        for djj0 in (0, 2):
            for tl in range(4):
                a, b = tl % 2, tl // 2
                wetq[tl * 32:(tl + 1) * 32, qi * 100:(qi + 1) * 100] = \
                    wet_om[:, :, dii0 + a, djj0 + b].T
            qi += 1
    for gi, dii0 in enumerate((0, 2)):
        for a in range(2):
            wetq[a * 32:(a + 1) * 32, 400 + gi * 100:500 + gi * 100] = \
                wet_om[:, :, dii0 + a, 4].T

    with_ebias = bool(b_comp.any() or b_enc.any())

    in_maps = []
    for core in range(NCORES):
        b = core // 4
        h0 = (core % 4) * HSLICE
        xpad = np.zeros((C, ROWS, WP), dtype=np.float32)
        r_lo = max(0, h0 - 2)
        r_hi = min(H, h0 + HSLICE + 2)
        xpad[:, (r_lo - (h0 - 2)):(r_hi - (h0 - 2)), 2:2 + W] = x[b, :, r_lo:r_hi, :]

        xt2 = np.empty((ROWS, 20, 4, C), dtype=bf16)
        for b4 in range(4):
            xt2[:, :, b4, :] = xpad[:, :, b4 * 16:b4 * 16 + 20].transpose(1, 2, 0)

        m = {
            "xs": np.ascontiguousarray(xpad.reshape(2, 128, PADPOS)).astype(bf16),
            "xt2": xt2,
            "wpk": wpk.astype(bf16),
            "wet32": wet32.astype(bf16),
            "wetq": wetq.astype(bf16),
        }
        if with_ebias:
            # field[o, h, w] = b_enc[o] + sum over valid taps of w_enc.b_comp
            wb = np.einsum("omt,m->ot", we, b_comp).reshape(NK, 5, 5)
            field = np.zeros((NK, HSLICE, W), dtype=np.float32)
            for di in range(-2, 3):
                for dj in range(-2, 3):
                    hh = np.arange(h0, h0 + HSLICE)[:, None] + di
                    ww = np.arange(W)[None, :] + dj
                    valid = ((hh >= 0) & (hh < H) & (ww >= 0) & (ww < W))
                    field += (
                        wb[:, di + 2, dj + 2][:, None, None]
                        * valid[None].astype(np.float32)
                    )
            field += b_enc[:, None, None]
            f = field.reshape(NK, 8, 2, 4, 16)        # (o, tile, ro, b4, w)
            f = np.transpose(f, (2, 0, 4, 1, 3))      # (ro, o, w, tile, b4)
            m["ebias"] = np.ascontiguousarray(f.reshape(2, NK, 512))
        in_maps.append(m)
    return in_maps, with_ebias


TRACE = False
LAST_RESULT = None


def kernel(x, w_comp, b_comp, w_enc, b_enc):
    global LAST_RESULT
    from concourse.bass_utils import run_bass_kernel_spmd

    in_maps, with_ebias = _prep_inputs(x, w_comp, b_comp, w_enc, b_enc)
    nc = _get_program(with_ebias)
    res = run_bass_kernel_spmd(
        nc, in_maps, core_ids=list(range(NCORES)), trace=TRACE
    )
    LAST_RESULT = res
    out = np.empty((B, C, 2 * H, 2 * W), dtype=np.float32)
    for core in range(NCORES):
        b = core // 4
        h0 = (core % 4) * HSLICE
        o = np.asarray(res.results[core]["out"], dtype=np.float32)
        # rows n = (ro, w, r1, r2), cols = (g, b4, c)
        o = o.reshape(2, 16, 2, 2, 8, 4, 256)
        # -> (c, g, ro, r1, b4, w, r2)
        o = np.transpose(o, (6, 4, 0, 2, 5, 1, 3)).reshape(256, 32, 128)
        out[b, :, 2 * h0:2 * h0 + 32, :] = o
    return out


# revision 52
# speedup vs baseline: 4.6219x; 1.0392x over previous
"""CARAFE content-aware upsampling on 8 Trainium2 NeuronCores (Bass/Tile).

Problem: x[2,256,64,64], 1x1 compress conv (256->32), 5x5 encoder conv
(32->100), pixel-shuffle(r=2) + softmax over 25 taps, then dynamic-filter
reassembly: out[b,c,2h+r1,2w+r2] = sum_k x[b,c,h+di,w+dj] * softmax_w.

Sharding: pure data-parallel over (batch, 16-row H slices) -> 8 cores.

Per-core pipeline (DMA-dispatch-minimal redesign, all-bf16 matmuls at
1 cyc/row; rel-tol is 2e-2 so bf16 error ~0.5% is fine):
  - compress (1x1) on PE; a 4-way shifted copy of y1 on 128 partitions
    (y1stack) lets the encoder contract 2x2 tap blocks, so the 5x5 conv
    is 11 matmuls per row-parity instead of 25.
  - softmax stays channel-major; a [100,128] permutation matmul then
    regroups (25 taps x 4 subpixels) partitions to taps-only partitions
    with sub on the free axis (32-aligned PSUM slices), after a
    broadcast matmul + one DVE multiply fold in the 1/sum.
  - the banded 25-tap MAC operand is built via a DRAM round trip: the
    diagonal (partition+free coupled) strides live entirely on the DRAM
    side, so the whole scatter is 10 large DMAs into a pre-zeroed DRAM
    scratch (ExternalOutput buffers arrive zeroed = the band gaps) + 6
    row-chunk loads back, instead of 160 per-column SBUF scatters.
  - x windows arrive pre-transposed from the host (xt2, bf16) and are
    gathered into the [120, g*b4*c] xcall operand by 6 SWDGE DMAs.
  - MAC: 32 matmuls (stationary = band block, moving = xcall 256-chan
    slice), psum [128n, 256c] pairs, bf16 results upcast on the host.
"""

import sys

sys.path.insert(0, "/opt/trn_rl_repo")

import numpy as np

import concourse.bacc as bacc
import concourse.bass as bass
import concourse.tile as tile
from concourse import mybir
from concourse.ap import AP

F32 = mybir.dt.float32
F32R = mybir.dt.float32r
BF16 = mybir.dt.bfloat16

# geometry
B, C, H, W = 2, 256, 64, 64
RATIO, K_UP, C_MID, ENC_K = 2, 5, 32, 5
NK = RATIO * RATIO * K_UP * K_UP  # 100
HSLICE = 16                       # output source rows per core
ROWS = HSLICE + 4                 # with 2-row halo each side
WP = W + 4                        # padded width
PADPOS = ROWS * WP                # 1360
NCORES = 8

KDIM = 120                        # 6 rows x 20 cols window pixels
YF = 4096                         # ybig free dim: col = 32*n + blk
XCF = 8192                        # xcall free dim: (g, b4, c)
YMSF = 2048                       # yMs free dim: (w, sub, tb)
WPKW = 296                        # wpk cols: wcat 64 | sel 4 | Eall 128 | selt 100


def build_program(with_ebias: bool):
    nc = bacc.Bacc()
    xs_d = nc.declare_dram_parameter("xs", [2, 128, PADPOS], BF16, isOutput=False)
    xt2_d = nc.declare_dram_parameter("xt2", [ROWS, 20, 4, C], BF16, isOutput=False)
    wpk_d = nc.declare_dram_parameter("wpk", [128, WPKW], BF16, isOutput=False)
    wet_d = nc.declare_dram_parameter("wet32", [C_MID, 25 * NK], BF16, isOutput=False)
    wetq_d = nc.declare_dram_parameter("wetq", [128, 600], BF16, isOutput=False)
    if with_ebias:
        ebias_d = nc.declare_dram_parameter("ebias", [2, NK, 512], F32, isOutput=False)
    # band scratch; ExternalOutput buffers are pre-zeroed by the runtime,
    # which supplies the structural zeros between the band diagonals.
    ydram_d = nc.declare_dram_parameter("ydram", [KDIM, YF], BF16, isOutput=True)
    out_d = nc.declare_dram_parameter("out", [128, 8192], BF16, isOutput=True)

    with tile.TileContext(nc) as tc:
        # The diagonal DRAM scatter APs (partition+free coupled strides on
        # the DRAM side) confuse the byte-range race detector; dependency
        # generation itself is tensor-granular and conservative.
        tc.race_detector_enabled = False
        with tc.tile_pool(name="persist", bufs=1) as pp:
            # ---- input loads (split so compress can start early) ----
            wpk = pp.tile([128, WPKW], BF16, tag="wpk")
            nc.scalar.dma_start(wpk[:], wpk_d[:])

            xst = []
            for ct in range(2):
                t = pp.tile([128, PADPOS], BF16, name=f"xst{ct}", tag=f"xs{ct}")
                xst.append(t)
            for piece in ((0, 512), (512, PADPOS)):
                for ct in range(2):
                    nc.sync.dma_start(
                        xst[ct][:, piece[0]:piece[1]],
                        xs_d[ct, :, piece[0]:piece[1]],
                    )

            wetb = pp.tile([C_MID, 25 * NK], BF16, tag="wetb")
            nc.scalar.dma_start(wetb[:], wet_d[:])
            wetq = pp.tile([128, 600], BF16, tag="wetq")
            nc.scalar.dma_start(wetq[:], wetq_d[:])

            if with_ebias:
                ebias = []
                for ro in range(2):
                    t = pp.tile([NK, 512], F32, tag=f"ebias{ro}")
                    nc.scalar.dma_start(t[:], ebias_d[ro])
                    ebias.append(t)

            # xcall[rq*20+cq, (g, b4, c)] = x[c, 2g+rq, b4*16+cq] (padded
            # coords), gathered from the host-transposed xt2 on the SWDGE
            # queue (Pool engine) to keep HWDGE free.
            xcall = pp.tile([KDIM, XCF], BF16, tag="xcall")
            for rq in range(6):
                dst = AP(xcall.tensor, rq * 20 * XCF,
                         [[XCF, 20], [1024, 8], [1, 1024]])
                src = AP(xt2_d, rq * 20480,
                         [[1024, 20], [40960, 8], [1, 1024]])
                nc.gpsimd.dma_start(dst, src)

            # ---- compress conv y1[32, PADPOS] ----
            psCMP = tc.alloc_tile_pool(name="psCMP", bufs=2, space="PSUM")
            y1 = pp.tile([C_MID, PADPOS], BF16, tag="y1")
            # shifted stack alongside y1: y1stack[tl*32+m, p] = y1[m, p+s_tl]
            # with s_tl in {0, WP, 1, WP+1}, so a 2x2 tap block contracts
            # 128 partitions. Built per compress chunk straight from PSUM.
            STK = PADPOS - WP - 1
            SHIFTS = (0, WP, 1, WP + 1)
            y1stack = pp.tile([128, PADPOS], BF16, tag="y1stack")
            off = 0
            ci = 0
            while off < PADPOS:
                n = min(512, PADPOS - off)
                ps = psCMP.tile([C_MID, 512], F32, tag="cmp")
                for ct in range(2):
                    nc.tensor.matmul(
                        ps[:, :n],
                        wpk[:, ct * 32:(ct + 1) * 32],
                        xst[ct][:, off:off + n],
                        start=(ct == 0), stop=(ct == 1),
                    )
                eng = nc.vector if ci % 2 == 0 else nc.scalar
                if eng is nc.vector:
                    eng.tensor_copy(y1[:, off:off + n], ps[:, :n])
                else:
                    eng.copy(y1[:, off:off + n], ps[:, :n])
                # stack slices fed by this chunk (read from y1 SBUF so the
                # psum tile is released immediately): stack cols
                # [off-s, off+n-s) <- y1 cols [off, off+n)
                for tl, sh in enumerate(SHIFTS):
                    a = max(0, off - sh)
                    b = min(STK, off + n - sh)
                    if b <= a:
                        continue
                    dst = y1stack[tl * 32:(tl + 1) * 32, a:b]
                    src = y1[:, a + sh:b + sh]
                    if tl % 3 == 0:
                        nc.gpsimd.tensor_copy(dst, src)
                    elif tl % 3 == 1:
                        nc.vector.tensor_copy(dst, src)
                    else:
                        nc.scalar.copy(dst, src)
                off += n
                ci += 1


            psCMP.release()
            psENC = tc.alloc_tile_pool(name="psENC", bufs=2, space="PSUM")
            psS = tc.alloc_tile_pool(name="psS", bufs=1, space="PSUM")
            psRG = tc.alloc_tile_pool(name="psRG", bufs=2, space="PSUM")
            psB = tc.alloc_tile_pool(name="psB", bufs=2, space="PSUM")

            # ---- encoder conv + exp, per output-row parity ro ----
            # rhs columns stream in pos' = (w, tile, b4) order:
            # f = 32*w + 4*tile + b4  <->  (h = 2*tile + ro, wcol = 16*b4 + w)
            # 11 matmuls: 5 singles (dii=4, from y1), 2 pairs (djj=4, from
            # the {0,WP} stack rows), 4 quads (2x2 tap blocks, full stack).
            def encode(ro):
                ps = psENC.tile([NK, 512], F32, tag="enc")
                POSDIMS = [[1, 16], [2 * WP, 8], [16, 4]]
                groups = []
                for djj in range(5):    # singles: tap (dii=4, djj)
                    groups.append((y1.tensor, C_MID,
                                   (ro + 4) * WP + djj,
                                   wetb[:, (20 + djj) * NK:(21 + djj) * NK]))
                for gi, dii0 in enumerate((0, 2)):   # pairs: (dii0+a, djj=4)
                    groups.append((y1stack.tensor, 64,
                                   (ro + dii0) * WP + 4,
                                   wetq[0:64, 400 + gi * 100:500 + gi * 100]))
                qi = 0
                for dii0 in (0, 2):                  # quads
                    for djj0 in (0, 2):
                        groups.append((y1stack.tensor, 128,
                                       (ro + dii0) * WP + djj0,
                                       wetq[:, qi * 100:(qi + 1) * 100]))
                        qi += 1
                for i, (mt, cp, moff, statw) in enumerate(groups):
                    rhs = AP(mt, moff, [[PADPOS, cp]] + POSDIMS)
                    nc.tensor.matmul(
                        ps[:], statw, rhs,
                        start=(i == 0), stop=(i == len(groups) - 1),
                    )
                y2e = pp.tile([NK, 512], BF16, tag=f"y2e{ro}")
                if with_ebias:
                    nc.vector.scalar_tensor_tensor(
                        y2e[:], ps[:], 1.0, ebias[ro][:],
                        op0=mybir.AluOpType.mult, op1=mybir.AluOpType.add,
                    )
                    nc.scalar.activation(
                        y2e[:], y2e[:], mybir.ActivationFunctionType.Exp
                    )
                else:
                    nc.scalar.activation(
                        y2e[:], ps[:], mybir.ActivationFunctionType.Exp
                    )
                return y2e

            # tap-sum via select matmul [100,4] -> [4, 512], then reciprocal
            def tapsums(y2e, ro):
                ps = psS.tile([4, 512], F32, tag="sums")
                nc.tensor.matmul(ps[:], wpk[0:NK, 64:68],
                                 y2e[:], start=True, stop=True)
                rs4 = pp.tile([4, 512], BF16, tag=f"rs4{ro}")
                with nc.allow_low_precision(
                    reason="softmax 1/sum in bf16; rel-tol is 2e-2"
                ):
                    nc.vector.reciprocal(rs4[:], ps[:])
                return rs4

            # normalize (broadcast 1/sum over partitions via selt matmul,
            # one DVE multiply), then regroup to taps-on-partitions with
            # sub on the free axis via 4 select matmuls + copies:
            #   yMs[kk, 128*w + 32*sub + tb] = y2e[4*kk+sub, f] * rs4[sub, f]
            def regroup(y2e, rs4, ro, yms):
                pB = psB.tile([NK, 512], F32, tag="bc")
                nc.tensor.matmul(
                    pB[:], wpk[0:4, 196:296], rs4[:], start=True, stop=True,
                )
                ymn = pp.tile([NK, 512], BF16, name=f"ymn{ro}", tag=f"ymn{ro}")
                nc.vector.tensor_tensor(
                    ymn[:], y2e[:], pB[:], op=mybir.AluOpType.mult,
                )
                # one permutation matmul [100,100], then 4 psum-slice copies
                pG = psRG.tile([128, 512], F32, tag="rg")
                nc.tensor.matmul(
                    pG[:], wpk[0:NK, 68:196], ymn[:], start=True, stop=True,
                )
                for sub in range(4):
                    dst = AP(yms.tensor, sub * 32,
                             [[YMSF, 25], [128, 16], [1, 32]])
                    src = AP(pG.tensor, sub * 32 * 512,
                             [[512, 25], [32, 16], [1, 32]])
                    if sub % 2 == 0:
                        nc.vector.tensor_copy(dst, src)
                    else:
                        nc.scalar.copy(dst, src)

            # band scatter: per (ro, dii) one DMA into the DRAM scratch;
            # the w-diagonal (dst partition q = (ro+dii)*20 + w + djj AND
            # dst col 128*(16*ro + w) + ...) is a plain stride 4224 in flat
            # DRAM. Structural zeros come from the pre-zeroed output buf.
            def scatter(yms, ro):
                for dii in range(5):
                    src = AP(yms.tensor, dii * 5 * YMSF,
                             [[YMSF, 5], [128, 16], [1, 128]])
                    dst = AP(ydram_d,
                             (ro + dii) * 20 * YF + ro * 2048,
                             [[YF, 5], [YF + 128, 16], [1, 128]])
                    eng = (nc.sync, nc.gpsimd, nc.sync, nc.gpsimd,
                           nc.sync)[dii]
                    eng.dma_start(dst, src)

            yms0 = pp.tile([25, YMSF], BF16, tag="yms0")
            yms1 = pp.tile([25, YMSF], BF16, tag="yms1")

            y2e0 = encode(0)
            rs40 = tapsums(y2e0, 0)
            y2e1 = encode(1)          # PE busy while DVE does recip ro0
            regroup(y2e0, rs40, 0, yms0)
            scatter(yms0, 0)
            rs41 = tapsums(y2e1, 1)
            regroup(y2e1, rs41, 1, yms1)
            scatter(yms1, 1)

            for pool in (psB, psRG, psS, psENC):
                pool.release()
            psMAC = tc.alloc_tile_pool(name="psMAC", bufs=6, space="PSUM")

            # ---- band load back (row-chunked) + 25-tap MAC ----
            # psum [n, c]: stationary = ybig block (strided cols), moving =
            # xcall full-channel slice (256 rows/matmul, 1 mm per block).
            ybig = pp.tile([KDIM, YF], BF16, tag="ybig")
            for i in range(6):
                r0, r1 = i * 20, (i + 1) * 20
                eng = (nc.sync, nc.gpsimd)[i % 2]
                eng.dma_start(
                    AP(ybig.tensor, r0 * YF, [[YF, r1 - r0], [1, YF]]),
                    AP(ydram_d, r0 * YF, [[YF, r1 - r0], [1, YF]]),
                )

            osbs = [pp.tile([128, 2048], BF16, name=f"osb{i}", tag=f"osb{i}")
                    for i in range(4)]
            for k in range(16):             # 2 blocks per psum tile
                ps = psMAC.tile([128, 512], F32, tag="mac")
                for j in range(2):
                    blk = k * 2 + j
                    nc.tensor.matmul(
                        ps[:, j * 256:(j + 1) * 256],
                        AP(ybig.tensor, blk, [[YF, KDIM], [32, 128]]),
                        AP(xcall.tensor, blk * 256, [[XCF, KDIM], [1, 256]]),
                        start=True, stop=True,
                    )
                osb = osbs[k // 4]
                col = (k % 4) * 512
                if k % 2 == 0:
                    nc.vector.tensor_copy(osb[:, col:col + 512], ps[:])
                else:
                    nc.scalar.copy(osb[:, col:col + 512], ps[:])
                if k % 2 == 1:
                    # ship each half-quarter as soon as its copies land
                    # (SP/Pool queues so Act SEQ stays free for copies)
                    q, h = k // 4, (k % 4) // 2
                    eng = nc.sync if k % 4 == 1 else nc.gpsimd
                    eng.dma_start(
                        out_d[:, q * 2048 + h * 1024:q * 2048 + h * 1024 + 1024],
                        osbs[q][:, h * 1024:(h + 1) * 1024],
                    )
            psMAC.release()
    nc.compile()
    return nc


_CACHE: dict[bool, object] = {}


def _get_program(with_ebias: bool):
    if with_ebias not in _CACHE:
        _CACHE[with_ebias] = build_program(with_ebias)
    return _CACHE[with_ebias]


def _prep_inputs(x, w_comp, b_comp, w_enc, b_enc):
    """Build the per-core numpy input dicts."""
    import ml_dtypes

    bf16 = ml_dtypes.bfloat16
    x = np.asarray(x, dtype=np.float32)
    w_comp = np.asarray(w_comp, dtype=np.float32)
    b_comp = np.asarray(b_comp, dtype=np.float32)
    w_enc = np.asarray(w_enc, dtype=np.float32)
    b_enc = np.asarray(b_enc, dtype=np.float32)

    # packed weights: wcat | sel | Esel | esub
    wpk = np.zeros((128, WPKW), dtype=np.float32)
    for ct in range(2):
        wpk[:, ct * 32:(ct + 1) * 32] = w_comp[:, ct * 128:(ct + 1) * 128].T
    p = np.arange(NK)
    wpk[p, 64 + p % 4] = 1.0                       # sel
    wpk[p, 68 + (p % 4) * 32 + p // 4] = 1.0       # Eall (32-aligned subs)
    wpk[p % 4, 196 + p] = 1.0                      # selt

    we = w_enc.reshape(NK, C_MID, 25)              # [o, m, tap]
    wet32 = np.ascontiguousarray(
        np.transpose(we, (1, 2, 0)).reshape(C_MID, 25 * NK)
    )

    # wetq: stacked-encoder stationaries. Stack row tl = b*2 + a carries
    # shift a*WP + b, so quad (dii0, djj0) row (tl, m) = w_enc tap
    # (dii0+a, djj0+b); pairs cover (dii0+a, djj=4) on rows tl in {0,1}.
    wetq = np.zeros((128, 600), dtype=np.float32)
    wet_om = we.reshape(NK, C_MID, 5, 5)           # [o, m, dii, djj]
    qi = 0
    for dii0 in (0, 2):
        for djj0 in (0, 2):
            for tl in range(4):
                a, b = tl % 2, tl // 2
                wetq[tl * 32:(tl + 1) * 32, qi * 100:(qi + 1) * 100] = \
                    wet_om[:, :, dii0 + a, djj0 + b].T
            qi += 1
    for gi, dii0 in enumerate((0, 2)):
        for a in range(2):
            wetq[a * 32:(a + 1) * 32, 400 + gi * 100:500 + gi * 100] = \
                wet_om[:, :, dii0 + a, 4].T

    with_ebias = bool(b_comp.any() or b_enc.any())

    in_maps = []
    for core in range(NCORES):
        b = core // 4
        h0 = (core % 4) * HSLICE
        xpad = np.zeros((C, ROWS, WP), dtype=np.float32)
        r_lo = max(0, h0 - 2)
        r_hi = min(H, h0 + HSLICE + 2)
        xpad[:, (r_lo - (h0 - 2)):(r_hi - (h0 - 2)), 2:2 + W] = x[b, :, r_lo:r_hi, :]

        xt2 = np.empty((ROWS, 20, 4, C), dtype=bf16)
        for b4 in range(4):
            xt2[:, :, b4, :] = xpad[:, :, b4 * 16:b4 * 16 + 20].transpose(1, 2, 0)

        m = {
            "xs": np.ascontiguousarray(xpad.reshape(2, 128, PADPOS)).astype(bf16),
            "xt2": xt2,
            "wpk": wpk.astype(bf16),
            "wet32": wet32.astype(bf16),
            "wetq": wetq.astype(bf16),
        }
        if with_ebias:
            # field[o, h, w] = b_enc[o] + sum over valid taps of w_enc.b_comp
            wb = np.einsum("omt,m->ot", we, b_comp).reshape(NK, 5, 5)
            field = np.zeros((NK, HSLICE, W), dtype=np.float32)
            for di in range(-2, 3):
                for dj in range(-2, 3):
                    hh = np.arange(h0, h0 + HSLICE)[:, None] + di
                    ww = np.arange(W)[None, :] + dj
                    valid = ((hh >= 0) & (hh < H) & (ww >= 0) & (ww < W))
                    field += (
                        wb[:, di + 2, dj + 2][:, None, None]
                        * valid[None].astype(np.float32)
                    )
            field += b_enc[:, None, None]
            f = field.reshape(NK, 8, 2, 4, 16)        # (o, tile, ro, b4, w)
            f = np.transpose(f, (2, 0, 4, 1, 3))      # (ro, o, w, tile, b4)
            m["ebias"] = np.ascontiguousarray(f.reshape(2, NK, 512))
        in_maps.append(m)
    return in_maps, with_ebias


TRACE = False
LAST_RESULT = None


def kernel(x, w_comp, b_comp, w_enc, b_enc):
    global LAST_RESULT
    from concourse.bass_utils import run_bass_kernel_spmd

    in_maps, with_ebias = _prep_inputs(x, w_comp, b_comp, w_enc, b_enc)
    nc = _get_program(with_ebias)
    res = run_bass_kernel_spmd(
        nc, in_maps, core_ids=list(range(NCORES)), trace=TRACE
    )
    LAST_RESULT = res
    out = np.empty((B, C, 2 * H, 2 * W), dtype=np.float32)
    for core in range(NCORES):
        b = core // 4
        h0 = (core % 4) * HSLICE
        o = np.asarray(res.results[core]["out"], dtype=np.float32)
        # rows n = (ro, w, r1, r2), cols = (g, b4, c)
        o = o.reshape(2, 16, 2, 2, 8, 4, 256)
        # -> (c, g, ro, r1, b4, w, r2)
        o = np.transpose(o, (6, 4, 0, 2, 5, 1, 3)).reshape(256, 32, 128)
        out[b, :, 2 * h0:2 * h0 + 32, :] = o
    return out


# revision 63
# speedup vs baseline: 4.8099x; 1.0407x over previous
"""CARAFE content-aware upsampling on 8 Trainium2 NeuronCores (Bass/Tile).

Problem: x[2,256,64,64], 1x1 compress conv (256->32), 5x5 encoder conv
(32->100), pixel-shuffle(r=2) + softmax over 25 taps, then dynamic-filter
reassembly: out[b,c,2h+r1,2w+r2] = sum_k x[b,c,h+di,w+dj] * softmax_w.

Sharding: pure data-parallel over (batch, 16-row H slices) -> 8 cores.

Per-core pipeline (DMA-dispatch-minimal redesign, all-bf16 matmuls at
1 cyc/row; rel-tol is 2e-2 so bf16 error ~0.5% is fine):
  - compress (1x1) on PE; a 4-way shifted copy of y1 on 128 partitions
    (y1stack) lets the encoder contract 2x2 tap blocks, so the 5x5 conv
    is 11 matmuls per row-parity instead of 25.
  - softmax stays channel-major; a [100,128] permutation matmul then
    regroups (25 taps x 4 subpixels) partitions to taps-only partitions
    with sub on the free axis (32-aligned PSUM slices), after a
    broadcast matmul + one DVE multiply fold in the 1/sum.
  - the banded 25-tap MAC operand is built via a DRAM round trip: the
    diagonal (partition+free coupled) strides live entirely on the DRAM
    side, so the whole scatter is 10 large DMAs into a pre-zeroed DRAM
    scratch (ExternalOutput buffers arrive zeroed = the band gaps) + 6
    row-chunk loads back, instead of 160 per-column SBUF scatters.
  - x windows arrive pre-transposed from the host (xt2, bf16) and are
    gathered into the [120, g*b4*c] xcall operand by 6 SWDGE DMAs.
  - MAC: 32 matmuls (stationary = band block, moving = xcall 256-chan
    slice), psum [128n, 256c] pairs, bf16 results upcast on the host.
"""

import sys

sys.path.insert(0, "/opt/trn_rl_repo")

import numpy as np

import concourse.bacc as bacc
import concourse.bass as bass
import concourse.tile as tile
from concourse import mybir
from concourse.ap import AP

F32 = mybir.dt.float32
F32R = mybir.dt.float32r
BF16 = mybir.dt.bfloat16

# geometry
B, C, H, W = 2, 256, 64, 64
RATIO, K_UP, C_MID, ENC_K = 2, 5, 32, 5
NK = RATIO * RATIO * K_UP * K_UP  # 100
HSLICE = 16                       # output source rows per core
ROWS = HSLICE + 4                 # with 2-row halo each side
WP = W + 4                        # padded width
PADPOS = ROWS * WP                # 1360
NCORES = 8

KDIM = 120                        # 6 rows x 20 cols window pixels
YF = 4096                         # ybig free dim: col = 32*n + blk
XCF = 8192                        # xcall free dim: (g, b4, c)
YMSF = 2048                       # yMs free dim: (w, sub, tb)
WPKW = 296                        # wpk cols: wcat 64 | sel 4 | Eall 128 | selt 100


def build_program(with_ebias: bool):
    nc = bacc.Bacc()
    xs_d = nc.declare_dram_parameter("xs", [2, 128, PADPOS], BF16, isOutput=False)
    xt2_d = nc.declare_dram_parameter("xt2", [ROWS, 20, 4, C], BF16, isOutput=False)
    wpk_d = nc.declare_dram_parameter("wpk", [128, WPKW], BF16, isOutput=False)
    wet_d = nc.declare_dram_parameter("wet32", [C_MID, 25 * NK], BF16, isOutput=False)
    wetq_d = nc.declare_dram_parameter("wetq", [128, 600], BF16, isOutput=False)
    if with_ebias:
        ebias_d = nc.declare_dram_parameter("ebias", [2, NK, 512], F32, isOutput=False)
    # band scratch; ExternalOutput buffers are pre-zeroed by the runtime,
    # which supplies the structural zeros between the band diagonals.
    ydram_d = nc.declare_dram_parameter("ydram", [KDIM, YF], BF16, isOutput=True)
    out_d = nc.declare_dram_parameter("out", [128, 8192], BF16, isOutput=True)

    with tile.TileContext(nc) as tc:
        # The diagonal DRAM scatter APs (partition+free coupled strides on
        # the DRAM side) confuse the byte-range race detector; dependency
        # generation itself is tensor-granular and conservative.
        tc.race_detector_enabled = False
        with tc.tile_pool(name="persist", bufs=1) as pp:
            # ---- input loads (split so compress can start early) ----
            wpk = pp.tile([128, WPKW], BF16, tag="wpk")
            nc.scalar.dma_start(wpk[:], wpk_d[:])

            xst = []
            for ct in range(2):
                t = pp.tile([128, PADPOS], BF16, name=f"xst{ct}", tag=f"xs{ct}")
                xst.append(t)
            for piece in ((0, 512), (512, 1024), (1024, PADPOS)):
                for ct in range(2):
                    eng = nc.sync if ct == 0 else nc.scalar
                    eng.dma_start(
                        xst[ct][:, piece[0]:piece[1]],
                        xs_d[ct, :, piece[0]:piece[1]],
                    )

            wetb = pp.tile([C_MID, 25 * NK], BF16, tag="wetb")
            nc.scalar.dma_start(wetb[:], wet_d[:])
            wetq = pp.tile([128, 600], BF16, tag="wetq")
            nc.scalar.dma_start(wetq[:], wetq_d[:])

            if with_ebias:
                ebias = []
                for ro in range(2):
                    t = pp.tile([NK, 512], F32, tag=f"ebias{ro}")
                    nc.scalar.dma_start(t[:], ebias_d[ro])
                    ebias.append(t)

            # xcall[rq*20+cq, (g, b4, c)] = x[c, 2g+rq, b4*16+cq] (padded
            # coords), gathered from the host-transposed xt2 on the SWDGE
            # queue (Pool engine) to keep HWDGE free.
            xcall = pp.tile([KDIM, XCF], BF16, tag="xcall")
            for rq in range(6):
                dst = AP(xcall.tensor, rq * 20 * XCF,
                         [[XCF, 20], [1024, 8], [1, 1024]])
                src = AP(xt2_d, rq * 20480,
                         [[1024, 20], [40960, 8], [1, 1024]])
                nc.gpsimd.dma_start(dst, src)

            # ---- compress conv y1[32, PADPOS] ----
            psCMP = tc.alloc_tile_pool(name="psCMP", bufs=2, space="PSUM")
            y1 = pp.tile([C_MID, PADPOS], BF16, tag="y1")
            # shifted stack alongside y1: y1stack[tl*32+m, p] = y1[m, p+s_tl]
            # with s_tl in {0, WP, 1, WP+1}, so a 2x2 tap block contracts
            # 128 partitions. Built per compress chunk straight from PSUM.
            STK = PADPOS - WP - 1
            SHIFTS = (0, WP, 1, WP + 1)
            y1stack = pp.tile([128, PADPOS], BF16, tag="y1stack")
            off = 0
            ci = 0
            while off < PADPOS:
                n = min(512, PADPOS - off)
                ps = psCMP.tile([C_MID, 512], F32, tag="cmp")
                for ct in range(2):
                    nc.tensor.matmul(
                        ps[:, :n],
                        wpk[:, ct * 32:(ct + 1) * 32],
                        xst[ct][:, off:off + n],
                        start=(ct == 0), stop=(ct == 1),
                    )
                eng = nc.vector if ci % 2 == 0 else nc.scalar
                if eng is nc.vector:
                    eng.tensor_copy(y1[:, off:off + n], ps[:, :n])
                else:
                    eng.copy(y1[:, off:off + n], ps[:, :n])
                # stack slices fed by this chunk (read from y1 SBUF so the
                # psum tile is released immediately): stack cols
                # [off-s, off+n-s) <- y1 cols [off, off+n)
                for tl, sh in enumerate(SHIFTS):
                    a = max(0, off - sh)
                    b = min(STK, off + n - sh)
                    if b <= a:
                        continue
                    dst = y1stack[tl * 32:(tl + 1) * 32, a:b]
                    src = y1[:, a + sh:b + sh]
                    if tl % 3 == 0:
                        nc.gpsimd.tensor_copy(dst, src)
                    elif tl % 3 == 1:
                        nc.vector.tensor_copy(dst, src)
                    else:
                        nc.scalar.copy(dst, src)
                off += n
                ci += 1


            psCMP.release()
            psENC = tc.alloc_tile_pool(name="psENC", bufs=2, space="PSUM")
            psS = tc.alloc_tile_pool(name="psS", bufs=1, space="PSUM")
            psRG = tc.alloc_tile_pool(name="psRG", bufs=2, space="PSUM")
            psB = tc.alloc_tile_pool(name="psB", bufs=2, space="PSUM")

            # ---- encoder conv + exp, per output-row parity ro ----
            # rhs columns stream in pos' = (w, tile, b4) order:
            # f = 32*w + 4*tile + b4  <->  (h = 2*tile + ro, wcol = 16*b4 + w)
            # 11 matmuls: 5 singles (dii=4, from y1), 2 pairs (djj=4, from
            # the {0,WP} stack rows), 4 quads (2x2 tap blocks, full stack).
            def encode(ro):
                ps = psENC.tile([NK, 512], F32, tag="enc")
                POSDIMS = [[1, 16], [2 * WP, 8], [16, 4]]
                groups = []
                for djj in range(5):    # singles: tap (dii=4, djj)
                    groups.append((y1.tensor, C_MID,
                                   (ro + 4) * WP + djj,
                                   wetb[:, (20 + djj) * NK:(21 + djj) * NK]))
                for gi, dii0 in enumerate((0, 2)):   # pairs: (dii0+a, djj=4)
                    groups.append((y1stack.tensor, 64,
                                   (ro + dii0) * WP + 4,
                                   wetq[0:64, 400 + gi * 100:500 + gi * 100]))
                qi = 0
                for dii0 in (0, 2):                  # quads
                    for djj0 in (0, 2):
                        groups.append((y1stack.tensor, 128,
                                       (ro + dii0) * WP + djj0,
                                       wetq[:, qi * 100:(qi + 1) * 100]))
                        qi += 1
                for i, (mt, cp, moff, statw) in enumerate(groups):
                    rhs = AP(mt, moff, [[PADPOS, cp]] + POSDIMS)
                    nc.tensor.matmul(
                        ps[:], statw, rhs,
                        start=(i == 0), stop=(i == len(groups) - 1),
                    )
                y2e = pp.tile([NK, 512], BF16, tag=f"y2e{ro}")
                if with_ebias:
                    nc.vector.scalar_tensor_tensor(
                        y2e[:], ps[:], 1.0, ebias[ro][:],
                        op0=mybir.AluOpType.mult, op1=mybir.AluOpType.add,
                    )
                    nc.scalar.activation(
                        y2e[:], y2e[:], mybir.ActivationFunctionType.Exp
                    )
                else:
                    nc.scalar.activation(
                        y2e[:], ps[:], mybir.ActivationFunctionType.Exp
                    )
                return y2e

            # tap-sum via select matmul [100,4] -> [4, 512], then reciprocal
            def tapsums(y2e, ro):
                ps = psS.tile([4, 512], F32, tag="sums")
                nc.tensor.matmul(ps[:], wpk[0:NK, 64:68],
                                 y2e[:], start=True, stop=True)
                rs4 = pp.tile([4, 512], BF16, tag=f"rs4{ro}")
                with nc.allow_low_precision(
                    reason="softmax 1/sum in bf16; rel-tol is 2e-2"
                ):
                    nc.vector.reciprocal(rs4[:], ps[:])
                return rs4

            # normalize (broadcast 1/sum over partitions via selt matmul,
            # one DVE multiply), then regroup to taps-on-partitions with
            # sub on the free axis via 4 select matmuls + copies:
            #   yMs[kk, 128*w + 32*sub + tb] = y2e[4*kk+sub, f] * rs4[sub, f]
            def regroup(y2e, rs4, ro, yms):
                pB = psB.tile([NK, 512], F32, tag="bc")
                nc.tensor.matmul(
                    pB[:], wpk[0:4, 196:296], rs4[:], start=True, stop=True,
                )
                ymn = pp.tile([NK, 512], BF16, name=f"ymn{ro}", tag=f"ymn{ro}")
                nc.vector.tensor_tensor(
                    ymn[:], y2e[:], pB[:], op=mybir.AluOpType.mult,
                )
                # one permutation matmul [100,100], then 4 psum-slice copies
                pG = psRG.tile([128, 512], F32, tag="rg")
                nc.tensor.matmul(
                    pG[:], wpk[0:NK, 68:196], ymn[:], start=True, stop=True,
                )
                for sub in range(4):
                    dst = AP(yms.tensor, sub * 32,
                             [[YMSF, 25], [128, 16], [1, 32]])
                    src = AP(pG.tensor, sub * 32 * 512,
                             [[512, 25], [32, 16], [1, 32]])
                    if sub % 2 == 0:
                        nc.vector.tensor_copy(dst, src)
                    else:
                        nc.scalar.copy(dst, src)

            # band scatter: per (ro, dii) one DMA into the DRAM scratch;
            # the w-diagonal (dst partition q = (ro+dii)*20 + w + djj AND
            # dst col 128*(16*ro + w) + ...) is a plain stride 4224 in flat
            # DRAM. Structural zeros come from the pre-zeroed output buf.
            def scatter(yms, ro):
                # ro0 avoids the Act queue (the ro1 regroup copies are
                # dispatched behind it); ro1 may use Act and spreads over
                # 3 queues so its last pieces land sooner.
                engs = (nc.sync, nc.gpsimd, nc.sync, nc.gpsimd, nc.sync)
                for dii in range(5):
                    src = AP(yms.tensor, dii * 5 * YMSF,
                             [[YMSF, 5], [128, 16], [1, 128]])
                    dst = AP(ydram_d,
                             (ro + dii) * 20 * YF + ro * 2048,
                             [[YF, 5], [YF + 128, 16], [1, 128]])
                    engs[dii].dma_start(dst, src)

            yms0 = pp.tile([25, YMSF], BF16, tag="yms0")
            yms1 = pp.tile([25, YMSF], BF16, tag="yms1")

            y2e0 = encode(0)
            rs40 = tapsums(y2e0, 0)
            y2e1 = encode(1)          # PE busy while DVE does recip ro0
            regroup(y2e0, rs40, 0, yms0)
            scatter(yms0, 0)
            rs41 = tapsums(y2e1, 1)
            regroup(y2e1, rs41, 1, yms1)
            scatter(yms1, 1)

            for pool in (psB, psRG, psS, psENC):
                pool.release()
            psMAC = tc.alloc_tile_pool(name="psMAC", bufs=6, space="PSUM")

            # ---- band load back (row-chunked) + 25-tap MAC ----
            # psum [n, c]: stationary = ybig block (strided cols), moving =
            # xcall full-channel slice (256 rows/matmul, 1 mm per block).
            ybig = pp.tile([KDIM, YF], BF16, tag="ybig")
            for i in range(6):
                r0, r1 = i * 20, (i + 1) * 20
                eng = (nc.gpsimd, nc.sync)[i % 2]
                eng.dma_start(
                    AP(ybig.tensor, r0 * YF, [[YF, r1 - r0], [1, YF]]),
                    AP(ydram_d, r0 * YF, [[YF, r1 - r0], [1, YF]]),
                )

            osbs = [pp.tile([128, 2048], BF16, name=f"osb{i}", tag=f"osb{i}")
                    for i in range(4)]
            for k in range(16):             # 2 blocks per psum tile
                ps = psMAC.tile([128, 512], F32, tag="mac")
                for j in range(2):
                    blk = k * 2 + j
                    nc.tensor.matmul(
                        ps[:, j * 256:(j + 1) * 256],
                        AP(ybig.tensor, blk, [[YF, KDIM], [32, 128]]),
                        AP(xcall.tensor, blk * 256, [[XCF, KDIM], [1, 256]]),
                        start=True, stop=True,
                    )
                osb = osbs[k // 4]
                col = (k % 4) * 512
                if k % 2 == 0:
                    nc.vector.tensor_copy(osb[:, col:col + 512], ps[:])
                else:
                    nc.scalar.copy(osb[:, col:col + 512], ps[:])
                if k % 2 == 1:
                    # ship each half-quarter as soon as its copies land
                    # (SP/Pool queues so Act SEQ stays free for copies);
                    # the final piece is split across two queues.
                    q, h = k // 4, (k % 4) // 2
                    base = q * 2048 + h * 1024
                    if k < 15:
                        eng = nc.gpsimd if k % 4 == 1 else nc.sync
                        eng.dma_start(
                            out_d[:, base:base + 1024],
                            osbs[q][:, h * 1024:h * 1024 + 1024],
                        )
                    else:
                        nc.sync.dma_start(
                            out_d[:, base:base + 512],
                            osbs[q][:, h * 1024:h * 1024 + 512],
                        )
                        nc.scalar.dma_start(
                            out_d[:, base + 512:base + 1024],
                            osbs[q][:, h * 1024 + 512:h * 1024 + 1024],
                        )
            psMAC.release()
    nc.compile()
    return nc


_CACHE: dict[bool, object] = {}


def _get_program(with_ebias: bool):
    if with_ebias not in _CACHE:
        _CACHE[with_ebias] = build_program(with_ebias)
    return _CACHE[with_ebias]


def _prep_inputs(x, w_comp, b_comp, w_enc, b_enc):
    """Build the per-core numpy input dicts."""
    import ml_dtypes

    bf16 = ml_dtypes.bfloat16
    x = np.asarray(x, dtype=np.float32)
    w_comp = np.asarray(w_comp, dtype=np.float32)
    b_comp = np.asarray(b_comp, dtype=np.float32)
    w_enc = np.asarray(w_enc, dtype=np.float32)
    b_enc = np.asarray(b_enc, dtype=np.float32)

    # packed weights: wcat | sel | Esel | esub
    wpk = np.zeros((128, WPKW), dtype=np.float32)
    for ct in range(2):
        wpk[:, ct * 32:(ct + 1) * 32] = w_comp[:, ct * 128:(ct + 1) * 128].T
    p = np.arange(NK)
    wpk[p, 64 + p % 4] = 1.0                       # sel
    wpk[p, 68 + (p % 4) * 32 + p // 4] = 1.0       # Eall (32-aligned subs)
    wpk[p % 4, 196 + p] = 1.0                      # selt

    we = w_enc.reshape(NK, C_MID, 25)              # [o, m, tap]
    wet32 = np.ascontiguousarray(
        np.transpose(we, (1, 2, 0)).reshape(C_MID, 25 * NK)
    )

    # wetq: stacked-encoder stationaries. Stack row tl = b*2 + a carries
    # shift a*WP + b, so quad (dii0, djj0) row (tl, m) = w_enc tap
    # (dii0+a, djj0+b); pairs cover (dii0+a, djj=4) on rows tl in {0,1}.
    wetq = np.zeros((128, 600), dtype=np.float32)
    wet_om = we.reshape(NK, C_MID, 5, 5)           # [o, m, dii, djj]
    qi = 0
    for dii0 in (0, 2):
        for djj0 in (0, 2):
            for tl in range(4):
                a, b = tl % 2, tl // 2
                wetq[tl * 32:(tl + 1) * 32, qi * 100:(qi + 1) * 100] = \
                    wet_om[:, :, dii0 + a, djj0 + b].T
            qi += 1
    for gi, dii0 in enumerate((0, 2)):
        for a in range(2):
            wetq[a * 32:(a + 1) * 32, 400 + gi * 100:500 + gi * 100] = \
                wet_om[:, :, dii0 + a, 4].T

    with_ebias = bool(b_comp.any() or b_enc.any())

    in_maps = []
    for core in range(NCORES):
        b = core // 4
        h0 = (core % 4) * HSLICE
        xpad = np.zeros((C, ROWS, WP), dtype=np.float32)
        r_lo = max(0, h0 - 2)
        r_hi = min(H, h0 + HSLICE + 2)
        xpad[:, (r_lo - (h0 - 2)):(r_hi - (h0 - 2)), 2:2 + W] = x[b, :, r_lo:r_hi, :]

        xt2 = np.empty((ROWS, 20, 4, C), dtype=bf16)
        for b4 in range(4):
            xt2[:, :, b4, :] = xpad[:, :, b4 * 16:b4 * 16 + 20].transpose(1, 2, 0)

        m = {
            "xs": np.ascontiguousarray(xpad.reshape(2, 128, PADPOS)).astype(bf16),
            "xt2": xt2,
            "wpk": wpk.astype(bf16),
            "wet32": wet32.astype(bf16),
            "wetq": wetq.astype(bf16),
        }
        if with_ebias:
            # field[o, h, w] = b_enc[o] + sum over valid taps of w_enc.b_comp
            wb = np.einsum("omt,m->ot", we, b_comp).reshape(NK, 5, 5)
            field = np.zeros((NK, HSLICE, W), dtype=np.float32)
            for di in range(-2, 3):
                for dj in range(-2, 3):
                    hh = np.arange(h0, h0 + HSLICE)[:, None] + di
                    ww = np.arange(W)[None, :] + dj
                    valid = ((hh >= 0) & (hh < H) & (ww >= 0) & (ww < W))
                    field += (
                        wb[:, di + 2, dj + 2][:, None, None]
                        * valid[None].astype(np.float32)
                    )
            field += b_enc[:, None, None]
            f = field.reshape(NK, 8, 2, 4, 16)        # (o, tile, ro, b4, w)
            f = np.transpose(f, (2, 0, 4, 1, 3))      # (ro, o, w, tile, b4)
            m["ebias"] = np.ascontiguousarray(f.reshape(2, NK, 512))
        in_maps.append(m)
    return in_maps, with_ebias


TRACE = False
LAST_RESULT = None


def kernel(x, w_comp, b_comp, w_enc, b_enc):
    global LAST_RESULT
    from concourse.bass_utils import run_bass_kernel_spmd

    in_maps, with_ebias = _prep_inputs(x, w_comp, b_comp, w_enc, b_enc)
    nc = _get_program(with_ebias)
    res = run_bass_kernel_spmd(
        nc, in_maps, core_ids=list(range(NCORES)), trace=TRACE
    )
    LAST_RESULT = res
    out = np.empty((B, C, 2 * H, 2 * W), dtype=np.float32)
    for core in range(NCORES):
        b = core // 4
        h0 = (core % 4) * HSLICE
        o = np.asarray(res.results[core]["out"], dtype=np.float32)
        # rows n = (ro, w, r1, r2), cols = (g, b4, c)
        o = o.reshape(2, 16, 2, 2, 8, 4, 256)
        # -> (c, g, ro, r1, b4, w, r2)
        o = np.transpose(o, (6, 4, 0, 2, 5, 1, 3)).reshape(256, 32, 128)
        out[b, :, 2 * h0:2 * h0 + 32, :] = o
    return out
